# revision 1
# baseline (speedup 1.0000x reference)
"""
Trainium2 Bass kernel for CondConv mask head (CondInst-style dynamic mask head).

Computation (for the fixed problem size):
  mask_feats (2, 8, 136, 200), 128 instances with per-instance 169 params
  -> per-instance 3-layer 1x1 convs over [rel_coords(2); feats(8)] -> (128,1,136,200)
  -> aligned_bilinear x2 upsample -> sigmoid -> (128, 1, 272, 400)

Strategy (8 NeuronCores, 16 instances per core):
  * Host folds the per-instance rel-coordinate channels into a shared 19-row
    spatial matrix Z = [locs_x; locs_y; ones; feats_im0(8); feats_im1(8)] and a
    per-(instance,outchan) lhsT A0T (19, 128); the c0 constant term rides on
    the ones-row.  Layer 1 is a block-diagonal lhsT (128,128).  All matmuls
    run in float32r (1 PE cycle/col); dummy warm-up matmuls bridge the PE
    p-state ramp while the first z DMA is in flight.
  * The image is processed in eight column-phases (widths 27x4, 23x4, each
    carrying one leading overlap column so phases are independent).  Each
    phase is 8 units; unit g covers rows 17g..17g+17 of the strip.
  * Per unit: mm0 -> PSUM -> relu evict (y0), mm1 -> PSUM -> relu+b1 evict
    (y1), and mm2 accumulates the unit's logits into a shared per-phase PSUM
    tile py2[128, 17*(W'+1)] at partitions 16g..16g+16 (partition = 16*block
    + inst) using a zero-padded [128,128] lhsT (w2 block at columns 16g,
    sliced from one 240-wide host-packed strip) -- a plain full matmul per
    unit, so the blocked layout costs no extra PE cycles and the upsample
    gets all 128 lanes with no re-partition DMA and no y2 eviction.
    b2 is folded into the sigmoid bias operand.
  * Evictions read PSUM so they may only run on ScalarE/VectorE (GPSIMD is
    PSUM-forbidden on HW); they are interleaved with ScalarE taking units
    {0,1,4,6} of each phase (emitted ahead of the upsample chain in its
    depth-0 queue).  One ScalarE activation copies py2 -> SBUF bf16 (sy);
    GpSimd then builds the odd out-cols (2x) and the C pass (adjacent wout
    sums), VectorE the even out-cols, all in SBUF bf16.
  * Block halo rows (wout 17g-1) ship via one partition-shifted SBUF->SBUF
    DMA per phase (fx slot 34 -> slot 0 of partition+16); block 0's halo is
    an edge-pad copy.  Sigmoids write out-of-place (scale 0.5/0.25 folds the
    2x/4x factors, bias=b2) into a contiguous bf16 buffer, deferred to a
    second emission stage so their data waits never block evictions.
  * Output is written bf16 in [block, inst, phase, 34, 2W'] layout (one
    128-descriptor contiguous DMA per phase); the host reassembles the
    strips and casts to float32.
"""

import os
import numpy as np

CH = 8
CIN = 8
N_IMG, H, W = 2, 136, 200
HW = H * W                      # 27200
N_INST = 128
N_CORES = 8
IPC = 16                        # instances per core
FACTOR = 2
OH, OW = H * FACTOR, W * FACTOR  # 272, 400
BLK = 8                         # row-blocks (= units) per phase
RPB = H // BLK                  # 17 rows per block
K0 = 3 + N_IMG * CIN            # 19 contraction rows for layer 0

PHW = [int(x) for x in os.environ.get(
    "K_PHW", "27,27,27,27,27,27,19,19").split(",")]  # W-phase widths
NPH = len(PHW)
PHOFF = np.cumsum([0] + PHW).tolist()
WMAX = max(PHW)
# each phase carries one extra leading (overlap) column so it is
# self-contained: unit g = rows 17g..17g+17 x (W'+1) columns
UMAX = RPB * (WMAX + 1)         # 476 cols <= one PSUM bank
N_UNITS = NPH * BLK             # 64
ZOFF = np.cumsum([0] + [H * (w + 1) for w in PHW]).tolist()
# packed per-phase output offsets (34 out rows x 2W' each, bf16)
OOFF = np.cumsum([0] + [2 * RPB * 2 * w for w in PHW]).tolist()

N_WARM = int(os.environ.get("K_WARM", "10"))  # PE warm-up matmuls
UPS_DELAY = int(os.environ.get("K_UPSD", "5"))  # upsample emission delay

LAST_EXEC_TIME_NS = None
_CACHE = {}


def _build_program():
    import concourse.bass as bass
    import concourse.bacc as bacc
    import concourse.tile as tile
    from concourse import mybir
    from contextlib import ExitStack

    f32 = mybir.dt.float32
    f32r = mybir.dt.float32r
    bf16 = mybir.dt.bfloat16
    Alu = mybir.AluOpType
    Act = mybir.ActivationFunctionType

    nc = bacc.Bacc("TRN2", target_bir_lowering=False, debug=False)

    zd = nc.dram_tensor("z_in", [K0, ZOFF[NPH]], f32r, kind="ExternalInput").ap()
    a0d = nc.dram_tensor("a0t_in", [K0, 128], f32r, kind="ExternalInput").ap()
    wpd = nc.dram_tensor("wpack_in", [128, 371], f32r, kind="ExternalInput").ap()
    # [block, inst, packed phase strips]: per phase the (34 x 2W') strip is
    # contiguous per (block, inst), so every out-DMA runs at full descriptor
    # width, and (block, inst) merge into the 128-partition dim
    outd = nc.dram_tensor("out", [BLK, IPC, OOFF[NPH]], bf16,
                          kind="ExternalOutput").ap()

    with tile.TileContext(nc) as tc, ExitStack() as ctx:
        consts = ctx.enter_context(tc.tile_pool(name="consts", bufs=1))
        a0t = consts.tile([K0, 128], f32r)
        wp = consts.tile([128, 371], f32r)
        warmz = consts.tile([K0, 256], f32)

        scr = consts.tile([1, 8], f32)

        # warm-up source must be initialized before the PE touches it
        nc.vector.memset(warmz[:], 0.0)

        # first z chunks (phase 0, two units each) then consts, then the rest
        zs = ctx.enter_context(tc.tile_pool(name="zs", bufs=1))
        zc = ctx.enter_context(tc.tile_pool(name="zc", bufs=2))
        z0 = zs.tile([K0, H * (PHW[0] + 1)], f32r, tag="z0")
        CH0 = 2 * RPB * (PHW[0] + 1)
        nc.sync.dma_start(z0[:, 0:CH0], zd[:, 0:CH0])
        nc.scalar.dma_start(a0t[:], a0d)
        nc.scalar.dma_start(wp[:], wpd)
        for q in range(1, 4):
            nc.sync.dma_start(z0[:, q * CH0:(q + 1) * CH0],
                              zd[:, q * CH0:(q + 1) * CH0])

        a0r = a0t[:]
        w1r = wp[:, 0:128]
        # cols 128:368 are a 240-wide zero strip with the w2 block-diagonal
        # at cols 240:256; the eight overlapping 128-wide windows place w2
        # at lhsT columns 16g, so each block's mm2 is a standard full
        # 128x128 matmul that accumulates zeros into the other blocks
        w2g = [wp[:, 240 - 16 * g:368 - 16 * g] for g in range(BLK)]
        b1ap = wp[:, 368:369].bitcast(f32)
        b2ap = wp[:, 369:370].bitcast(f32)

        y0p = ctx.enter_context(tc.tile_pool(name="y0p", bufs=4))
        y1p = ctx.enter_context(tc.tile_pool(name="y1p", bufs=6))
        PB = [int(c) for c in os.environ.get("K_PSUM", "233")]
        p0 = ctx.enter_context(tc.tile_pool(name="p0", bufs=PB[0], space="PSUM"))
        p1 = ctx.enter_context(tc.tile_pool(name="p1", bufs=PB[1], space="PSUM"))
        py2 = ctx.enter_context(tc.tile_pool(name="py2", bufs=PB[2], space="PSUM"))
        UB = int(os.environ.get("K_UB", "3"))
        fxp = ctx.enter_context(tc.tile_pool(name="fxp", bufs=UB))
        fop = ctx.enter_context(tc.tile_pool(name="fop", bufs=UB))
        syp = ctx.enter_context(tc.tile_pool(name="syp", bufs=UB))

        # absorb the activation-table loads during the idle start
        nc.scalar.activation(scr[:], warmz[0:1, 0:8], Act.Sigmoid)
        nc.scalar.activation(scr[:], warmz[0:1, 0:8], Act.Relu)

        # PE warm-up: keep the tensor engine busy (and ramping) while the
        # first z chunk's DMA completes.  Results are garbage and unread.
        for wi in range(N_WARM):
            pw = p0.tile([128, UMAX], f32, tag="ps0", name="pw")
            nc.tensor.matmul(pw[:, 0:256], warmz[:, 0:128].bitcast(f32r),
                             warmz[:].bitcast(f32r), start=True, stop=True)

        # evictions read PSUM, which only ScalarE/VectorE may touch on HW
        # (GPSIMD is PSUM-forbidden); GpSimd instead carries the SBUF-side
        # C pass and copies
        def evict(eng, dst, src, bias_ap):
            if eng == "act":
                if bias_ap is None:
                    nc.scalar.activation(dst, src, Act.Relu)
                else:
                    nc.scalar.activation(dst, src, Act.Relu, bias=bias_ap)
            else:
                if bias_ap is None:
                    nc.vector.tensor_scalar(dst, src, 0.0, None, Alu.max)
                else:
                    nc.vector.tensor_scalar(dst, src, bias_ap, 0.0,
                                            Alu.add, Alu.max)

        EV0 = tuple(int(x) for x in os.environ.get("K_EV0", "0,1,4,6").split(","))
        EV1 = tuple(int(x) for x in os.environ.get("K_EV1", "0,1,4,6").split(","))

        if os.environ.get("K_SY", "dve") == "dve":
            SY_ENG = nc.vector.tensor_copy
        else:
            SY_ENG = lambda d, s: nc.scalar.activation(d, s, Act.Identity)

        phase_state = {}

        def start_phase(p):
            if p + 1 < NPH:
                ncols = H * (PHW[p + 1] + 1)
                zt = zc.tile([K0, H * (WMAX + 1)], f32r, tag="z", name="zt")
                nc.scalar.dma_start(zt[:, 0:ncols],
                                    zd[:, ZOFF[p + 1]:ZOFF[p + 1] + ncols])
                phase_state[p + 1] = {"z": zt}
            st = phase_state.setdefault(p, {})
            if p == 0:
                st["z"] = z0
            st["py2"] = py2.tile([128, UMAX], f32, tag="py2", name="py2t")
            return st

        def s_mm0(i):
            p, g = divmod(i, BLK)
            st = phase_state.get(p)
            if g == 0:
                st = start_phase(p)
            uc = RPB * (PHW[p] + 1)
            zt = st["z"]
            p0t = p0.tile([128, UMAX], f32, tag="ps0", name="p0t")
            nc.tensor.matmul(p0t[:, 0:uc], a0r, zt[:, g * uc:(g + 1) * uc],
                             start=True, stop=True)
            y0t = y0p.tile([128, UMAX], f32r, tag="y0", name="y0t")
            eng = "act" if g in EV0 else "dve"
            evict(eng, y0t[:, 0:uc], p0t[:, 0:uc], None)
            st.setdefault("y0", {})[g] = y0t

        def s_mm1(i):
            p, g = divmod(i, BLK)
            st = phase_state[p]
            uc = RPB * (PHW[p] + 1)
            y0t = st["y0"].pop(g)
            p1t = p1.tile([128, UMAX], f32, tag="ps1", name="p1t")
            nc.tensor.matmul(p1t[:, 0:uc], w1r, y0t[:, 0:uc],
                             start=True, stop=True)
            y1t = y1p.tile([128, UMAX], f32r, tag="y1", name="y1t")
            eng = "act" if g in EV1 else "dve"
            evict(eng, y1t[:, 0:uc], p1t[:, 0:uc], b1ap)
            st.setdefault("y1", {})[g] = y1t

        def s_mm2(i):
            p, g = divmod(i, BLK)
            st = phase_state[p]
            uc = RPB * (PHW[p] + 1)
            y1t = st["y1"].pop(g)
            py2t = st["py2"]
            nc.tensor.matmul(py2t[:, 0:uc], w2g[g], y1t[:, 0:uc],
                             start=(g == 0), stop=(g == BLK - 1),
                             skip_group_check=True)
            if g == BLK - 1:
                pending_ups.append((p, i))

        def emit_upsample(p):
            st = phase_state[p]
            Wp = PHW[p]
            W1 = Wp + 1             # stored columns incl. the overlap col
            W2 = 2 * Wp
            py2t = st["py2"]
            # one PSUM->SBUF bf16 copy of the phase logits; A/B then run from
            # SBUF, where GpSimd may work and VectorE gets its 2-byte mode.
            # sy col 0 = spatial col -1 (overlap); col 1+j = spatial col j
            syh = syp.tile([128, RPB * (WMAX + 1)], bf16, tag="sy", name="syh")
            SY_ENG(syh[:, 0:RPB * W1], py2t[:, 0:RPB * W1])
            sy3 = syh[:, 0:RPB * W1].rearrange("q (r c) -> q r c", r=RPB)
            fxh = fxp.tile([128, 35 * 2 * WMAX], bf16, tag="fx", name="fxh")
            fx3 = fxh[:, 0:35 * W2].rearrange("q (v c) -> q v c", v=35)
            foh = fop.tile([128, 34 * 2 * WMAX], bf16, tag="fo", name="foh")
            fo3 = foh[:, 0:34 * W2].rearrange("q (v c) -> q v c", v=34)
            # wout row 16 (slot 34) first, so the halo shift DMA can launch
            # while the remaining wout rows are still being computed
            nc.gpsimd.tensor_scalar(fx3[:, 34:35, 1:W2:2], sy3[:, 16:17, 1:W1],
                                    2.0, None, Alu.mult)
            nc.gpsimd.tensor_tensor(fx3[:, 34:35, 0:W2 - 1:2],
                                     sy3[:, 16:17, 0:Wp], sy3[:, 16:17, 1:W1],
                                     Alu.add)
            # halo wout rows into slot 0 via partition-shifted SBUF DMA:
            # block g's wout(-1) = block g-1's wout(16) (fx slot 34)
            nc.scalar.dma_start(fx3[16:128, 0:1, :], fx3[0:112, 34:35, :])
            # A: odd out cols 2j+1 = 2*y2[j] (wout rows, stored 2x)
            nc.gpsimd.tensor_scalar(fx3[:, 2:34:2, 1:W2:2], sy3[:, 0:16, 1:W1],
                                    2.0, None, Alu.mult)
            # B: even out cols 2j = y2[j-1] + y2[j] (2x the interpolated value)
            nc.gpsimd.tensor_tensor(fx3[:, 2:34:2, 0:W2 - 1:2],
                                     sy3[:, 0:16, 0:Wp], sy3[:, 0:16, 1:W1],
                                     Alu.add)
            # block 0's wout(-1) = its own wout(0) (fx slot 2, edge pad) --
            # same partitions, so a cheap engine copy instead of a DMA
            nc.gpsimd.tensor_copy(fx3[0:16, 0:1, :], fx3[0:16, 2:3, :])
            # sigmoids write out-of-place into fo so sig-wout can run while
            # the shift->C path is still in flight (C reads pre-sigmoid wout)
            nc.scalar.activation(fo3[:, 1:34:2, :], fx3[:, 2:35:2, :],
                                 Act.Sigmoid, bias=b2ap, scale=0.5)
            # C: even out rows = adjacent wout sums (4x); SBUF-only -> GpSimd
            nc.gpsimd.tensor_tensor(fx3[:, 1:34:2, :], fx3[:, 0:33:2, :],
                                    fx3[:, 2:35:2, :], Alu.add)
            st["fx"] = (fx3, fo3)

        def emit_upsample2(p):
            fx3, fo3 = phase_state[p]["fx"]
            W2 = 2 * PHW[p]
            nc.scalar.activation(fo3[:, 0:33:2, :], fx3[:, 1:34:2, :],
                                 Act.Sigmoid, bias=b2ap, scale=0.25)
            dst = outd[:, :, OOFF[p]:OOFF[p] + 2 * RPB * W2]
            nc.sync.dma_start(dst.rearrange("g i (v c) -> g i v c", v=2 * RPB),
                              fo3[:])

        pending_ups = []
        pending_ups2 = []
        D1 = int(os.environ.get("K_D1", "2"))
        D2 = int(os.environ.get("K_D2", "6"))
        UPS_DELAY2 = UPS_DELAY + int(os.environ.get("K_UPSD2", "6"))
        for i in range(N_UNITS + D2 + UPS_DELAY2 + 1):
            if i < N_UNITS:
                s_mm0(i)
            if 0 <= i - D1 < N_UNITS:
                s_mm1(i - D1)
            if 0 <= i - D2 < N_UNITS:
                s_mm2(i - D2)
            while pending_ups and (i - D2 - pending_ups[0][1] >= UPS_DELAY
                                   or i - D2 >= N_UNITS):
                p_, i_ = pending_ups.pop(0)
                emit_upsample(p_)
                pending_ups2.append((p_, i_))
            while pending_ups2 and (i - D2 - pending_ups2[0][1] >= UPS_DELAY2
                                    or i - D2 >= N_UNITS + 4):
                emit_upsample2(pending_ups2.pop(0)[0])

    nc.compile()
    return nc


def _host_prep(mask_feats, mask_head_params, locations, im_inds, fpn_levels,
               sizes_of_interest):
    mask_feats = np.asarray(mask_feats, dtype=np.float32)
    params = np.asarray(mask_head_params, dtype=np.float32)
    locations = np.asarray(locations, dtype=np.float32)
    im_inds = np.asarray(im_inds).astype(np.int64)
    fpn_levels = np.asarray(fpn_levels).astype(np.int64)
    soi_tab = np.asarray(sizes_of_interest, dtype=np.float32)

    w0 = params[:, 0:80].reshape(N_INST, CH, CIN + 2)
    w1 = params[:, 80:144].reshape(N_INST, CH, CH)
    w2 = params[:, 144:152].reshape(N_INST, 1, CH)
    b0 = params[:, 152:160]
    b1 = params[:, 160:168]
    b2 = params[:, 168:169]

    soi = soi_tab[fpn_levels]                                    # (128,)
    alpha = -w0[:, :, 0] / soi[:, None]                          # (128, 8)
    beta = -w0[:, :, 1] / soi[:, None]
    c0 = b0 + (w0[:, :, 0] * locations[:, 0:1]
               + w0[:, :, 1] * locations[:, 1:2]) / soi[:, None]
    wfeat = w0[:, :, 2:]                                         # (128, 8, 8)

    stride = 8
    xs = np.arange(W, dtype=np.float32) * stride + stride // 2
    ys = np.arange(H, dtype=np.float32) * stride + stride // 2
    locs_x = np.tile(xs, H)
    locs_y = np.repeat(ys, W)
    z = np.concatenate([locs_x[None], locs_y[None],
                        np.ones((1, HW), np.float32),
                        mask_feats.reshape(N_IMG * CIN, HW)], axis=0)
    # reorder spatial into the column-phase blocks, each with one leading
    # overlap column (phase 0 duplicates column 0 as its overlap)
    z3 = z.reshape(K0, H, W)
    strips = []
    for q in range(NPH):
        lo = max(PHOFF[q] - 1, 0)
        s = z3[:, :, lo:PHOFF[q] + PHW[q]]
        if q == 0:
            s = np.concatenate([s[:, :, 0:1], s], axis=2)
        strips.append(s.reshape(K0, H * (PHW[q] + 1)))
    z = np.ascontiguousarray(np.concatenate(strips, axis=1), dtype=np.float32)

    in_maps = []
    for c in range(N_CORES):
        a0 = np.zeros((K0, 128), np.float32)
        wpack = np.zeros((128, 371), np.float32)
        b1v = np.zeros(128, np.float32)
        for i in range(IPC):
            gi = IPC * c + i
            for o in range(CH):
                m = CH * i + o
                a0[0, m] = alpha[gi, o]
                a0[1, m] = beta[gi, o]
                a0[2, m] = c0[gi, o]
                base = 3 + CIN * int(im_inds[gi])
                a0[base:base + CIN, m] = wfeat[gi, o, :]
                wpack[CH * i:CH * i + CH, m] = w1[gi, o, :]
                b1v[m] = b1[gi, o]
            wpack[CH * i:CH * i + CH, 240 + i] = w2[gi, 0, :]
        for q in range(128):
            wpack[q, 368] = b1v[q]
            wpack[q, 369] = b2[IPC * c + (q % IPC), 0]
        in_maps.append({
            "z_in": z,
            "a0t_in": np.ascontiguousarray(a0),
            "wpack_in": np.ascontiguousarray(wpack),
        })
    return in_maps


def kernel(mask_feats, mask_head_params, locations, im_inds, fpn_levels,
           sizes_of_interest, mask_feat_stride):
    global LAST_EXEC_TIME_NS
    assert int(mask_feat_stride) == 8, "kernel hardcodes mask_feat_stride=8"

    from concourse.bass_utils import run_bass_kernel_spmd

    in_maps = _host_prep(mask_feats, mask_head_params, locations, im_inds,
                         fpn_levels, sizes_of_interest)

    if "nc" not in _CACHE:
        _CACHE["nc"] = _build_program()
    nc = _CACHE["nc"]

    res = run_bass_kernel_spmd(nc, in_maps, list(range(N_CORES)), trace=False)
    LAST_EXEC_TIME_NS = res.exec_time_ns

    out = np.empty((N_INST, 1, OH, OW), np.float32)
    for c in range(N_CORES):
        dev = np.asarray(res.results[c]["out"]).astype(np.float32)
        for p in range(NPH):
            W2 = 2 * PHW[p]
            strip = dev[:, :, OOFF[p]:OOFF[p + 1]] \
                .reshape(BLK, IPC, 2 * RPB, W2) \
                .transpose(1, 0, 2, 3).reshape(IPC, OH, W2)
            out[IPC * c:IPC * (c + 1), 0, :, 2 * PHOFF[p]:2 * PHOFF[p + 1]] \
                = strip
    return out



# revision 17
# speedup vs baseline: 1.1018x; 1.1018x over previous
"""
Trainium2 Bass kernel for CondConv mask head (CondInst-style dynamic mask head).

Computation (fixed problem size):
  mask_feats (2, 8, 136, 200), 128 instances with per-instance 169 params
  -> per-instance 3-layer 1x1 convs over [rel_coords(2); feats(8)] -> (128,1,136,200)
  -> aligned_bilinear x2 upsample -> sigmoid -> (128, 1, 272, 400)

Strategy (8 NeuronCores, 16 instances per core), v2:
  * All matmul operands are bf16 (1 PE cycle/col).  Host folds rel-coords
    into a shared 19-row spatial matrix Z = [x; y; 1; feats_im0; feats_im1]
    and per-core lhsTs (a0 with the c0 constant on the ones-row,
    block-diagonal w1, and a zero-padded w2 strip whose eight 128-wide
    windows place the w2 block at lhsT columns 16g).
  * The image is processed in 8 column-phases of tunable widths (default
    20,28x6,12 -- narrow last phase to shorten the drain tail); each phase
    is 8 row-block units.  Row blocks OVERLAP by one row (18 rows, width*18
    cols <= 504): block g covers image rows 17g-1 .. 17g+16 (block 0
    edge-pads by duplicating row 0 in Z), so the x2-upsample top halo row
    is computed locally and no cross-partition halo DMA is needed.
  * mm0 writes unit PAIRS into a 2-bank PSUM tile (outs at col 0 and 512 so
    each matmul stays within a bank); the pair is evicted with one relu
    instruction (2D access pattern).  mm1/evict are per-unit; mm2
    accumulates phase logits into py2[128 = 8 blk x 16 inst].  PSUM-reading
    evictions run on ScalarE/VectorE only (GPSIMD has no PSUM port); the
    Act/DVE split is schedule-tuned.  PSUM: p0 2x2 banks + p1 2 + py2 2 = 8.
  * Upsample = 4 polyphase planes, packed bf16 in SBUF (VectorE 2x/4x
    modes, GpSimd helps off the critical chain):
      sy   = py2 copy (packed [18 x W'])
      O_rc = 2*sy[1:18]                 (odd row, odd col)
      QQ   = sy[.,j-1]+sy[.,j] (18 rows; rows 1..17 are the O_re plane;
             col 0 reads the previous phase's last col, tiny separate op)
      O_er = sy[k]+sy[k+1]              (even row, odd col)
      O_ee = QQ[k]+QQ[k+1]              (even row, even col, 4x logit)
  * Sigmoids: one Act instruction over [O_rc|QQ|O_er] (all 2x the logit:
    scale 0.5, bias b2) and one over O_ee (scale 0.25), emitted several
    units later so Act's in-order queue never head-blocks on the planes ->
    contiguous bf16 fo tile -> one 128-descriptor DMA per phase into
    [blk, inst, phase-strip] DRAM.  Host interleaves the planes, f32-casts.
"""

import os
import numpy as np

CH = 8
CIN = 8
N_IMG, H, W = 2, 136, 200
HW = H * W
N_INST = 128
N_CORES = 8
IPC = 16                         # instances per core
FACTOR = 2
OH, OW = H * FACTOR, W * FACTOR  # 272, 400
BLK = 8                          # row-blocks (= units) per phase
RPB = H // BLK                   # 17 output rows per block
RPU = RPB + 1                    # 18 stored rows per unit (one overlap row)
K0 = 3 + N_IMG * CIN             # 19 contraction rows for layer 0

PHW = [int(x) for x in os.environ.get(
    "K_PHW", "12,28,28,28,28,28,28,20").split(",")]
NPH = len(PHW)
assert sum(PHW) == W and all(w <= 28 for w in PHW)
ZUC = [RPB * w for w in PHW]              # 17-row unit cols per phase
# unit 0 of each phase has 18 rows (leading duplicate of image row 0, the
# edge-pad halo for block 0's 18-row mm2 window)
ZOFF = np.cumsum([0] + [(H + 1) * w for w in PHW]).tolist()
ZHW = ZOFF[-1]                            # 27400
PSTRIP = [(3 * RPB + RPU) * w for w in PHW]   # 69*w out cols per phase
OOFF = np.cumsum([0] + PSTRIP).tolist()
N_UNITS = NPH * BLK

N_WARM = int(os.environ.get("K_WARM", "3"))
EV0A = int(os.environ.get("K_EV0A", "26"))   # of 32 pair evicts (y0) on Act
EV1A = int(os.environ.get("K_EV1A", "0"))   # of 64 single evicts (y1) on Act

LAST_EXEC_TIME_NS = None
_CACHE = {}


def _spread(n_act, total, skip=3):
    # spread n_act picks over [skip, total) so the first evictions (pipeline
    # fill, before Act's activation tables are loaded) go to DVE
    if n_act <= 0:
        return set()
    n_act = min(n_act, total - skip)
    return set((skip + np.arange(n_act) * (total - skip) // n_act).tolist())


def _build_program():
    import concourse.bass as bass
    import concourse.bacc as bacc
    import concourse.tile as tile
    from concourse import mybir
    from contextlib import ExitStack

    f32 = mybir.dt.float32
    bf16 = mybir.dt.bfloat16
    Alu = mybir.AluOpType
    Act = mybir.ActivationFunctionType

    nc = bacc.Bacc("TRN2", target_bir_lowering=False, debug=False)

    zd = nc.dram_tensor("z_in", [K0, ZHW], bf16, kind="ExternalInput").ap()
    a0d = nc.dram_tensor("a0t_in", [K0, 128], bf16, kind="ExternalInput").ap()
    w1d = nc.dram_tensor("w1_in", [128, 128], bf16, kind="ExternalInput").ap()
    w2d = nc.dram_tensor("w2_in", [128, 368], bf16, kind="ExternalInput").ap()
    bd = nc.dram_tensor("b_in", [128, 2], f32, kind="ExternalInput").ap()
    outd = nc.dram_tensor("out", [BLK, IPC, OOFF[-1]], bf16,
                          kind="ExternalOutput").ap()

    ACT0 = _spread(EV0A, N_UNITS // 2)
    ACT1 = _spread(EV1A, N_UNITS, skip=5)

    U1 = int(os.environ.get("K_U1", "1"))
    U2 = int(os.environ.get("K_U2", "9"))
    U3 = int(os.environ.get("K_U3", "12"))
    D_EV0 = int(os.environ.get("K_DEV0", "1"))
    D_MM1 = int(os.environ.get("K_DMM1", "3"))
    D_EV1 = int(os.environ.get("K_DEV1", "5"))
    D_MM2 = int(os.environ.get("K_DMM2", "7"))
    # engines for [QQ, O_er, O_ee, O_rc]: p=pool, d=dve
    UPS_ENG = os.environ.get("K_UPS", "pppd")
    SY_ACT = os.environ.get("K_SY", "act") == "act"

    with tile.TileContext(nc) as tc, ExitStack() as ctx:
        consts = ctx.enter_context(tc.tile_pool(name="consts", bufs=1))
        zt = consts.tile([K0, ZHW], bf16)
        a0 = consts.tile([K0, 128], bf16)
        w1 = consts.tile([128, 128], bf16)
        w2s = consts.tile([128, 368], bf16)
        bb = consts.tile([128, 2], f32)
        warm = consts.tile([K0, 512], bf16)
        scr = consts.tile([1, 8], f32)

        # activation-table preloads first: tiny memset, then one dummy
        # activation per function so the table loads absorb into the idle
        # start instead of blocking the first Act evictions
        nc.vector.memset(scr[:], 0.0)
        nc.scalar.activation(scr[:], scr[:], Act.Sigmoid)
        nc.scalar.activation(scr[:], scr[:], Act.Relu)
        nc.scalar.activation(scr[:], scr[:], Act.Identity)
        nc.vector.memset(warm[:], 0.0)

        # z for the first unit-pair first (mm0 starts ASAP), then consts,
        # then the rest of z
        CH0 = 2 * ZUC[0] + PHW[0]
        nc.sync.dma_start(zt[:, 0:CH0], zd[:, 0:CH0])
        nc.sync.dma_start(a0[:], a0d)
        nc.sync.dma_start(zt[:, CH0:ZOFF[1]], zd[:, CH0:ZOFF[1]])
        nc.sync.dma_start(bb[:], bd)
        nc.sync.dma_start(w1[:], w1d)
        nc.sync.dma_start(w2s[:], w2d)
        nc.sync.dma_start(zt[:, ZOFF[1]:ZOFF[4]], zd[:, ZOFF[1]:ZOFF[4]])
        nc.sync.dma_start(zt[:, ZOFF[4]:], zd[:, ZOFF[4]:])

        b1ap = bb[:, 0:1]
        b2ap = bb[:, 1:2]
        w2g = [w2s[:, 240 - 16 * g:368 - 16 * g] for g in range(BLK)]

        p0p = ctx.enter_context(tc.tile_pool(name="p0p", bufs=2, space="PSUM"))
        p1p = ctx.enter_context(tc.tile_pool(name="p1p", bufs=2, space="PSUM"))
        py2p = ctx.enter_context(tc.tile_pool(name="py2p", bufs=2, space="PSUM"))
        y0p = ctx.enter_context(tc.tile_pool(name="y0p", bufs=3))
        y1p = ctx.enter_context(tc.tile_pool(name="y1p", bufs=2))
        syp = ctx.enter_context(tc.tile_pool(name="syp", bufs=3))
        upp = ctx.enter_context(tc.tile_pool(name="upp", bufs=3))
        fop = ctx.enter_context(tc.tile_pool(name="fop", bufs=3))

        # PE warm-up while the first z chunk's DMA is in flight
        for wi in range(N_WARM):
            pw = p0p.tile([128, 1024], f32, tag="p0", name="pw")
            nc.tensor.matmul(pw[:, 0:512], warm[:, 0:128], warm[:],
                             start=True, stop=True)

        state = {}

        def evict(on_act, dst, src, bias_ap):
            if on_act:
                if bias_ap is None:
                    nc.scalar.activation(dst, src, Act.Relu)
                else:
                    nc.scalar.activation(dst, src, Act.Relu, bias=bias_ap)
            else:
                if bias_ap is None:
                    nc.vector.tensor_scalar(dst, src, 0.0, None, Alu.max)
                else:
                    nc.vector.tensor_scalar(dst, src, bias_ap, 0.0,
                                            Alu.add, Alu.max)

        def ucols(p, g):
            # unit 0 of a phase carries the extra duplicate row (18 rows)
            return ZUC[p] + (PHW[p] if g == 0 else 0)

        def uoff(p, g):
            return ZOFF[p] + (RPB * g + (1 if g else 0)) * PHW[p]

        def s_mm0(u):
            p, g = divmod(u, BLK)
            q = u // 2
            ua, ub = ucols(p, g), ucols(p, g + 1)
            p0t = p0p.tile([128, 1024], f32, tag="p0", name="p0t")
            nc.tensor.matmul(p0t[:, 0:ua], a0[:],
                             zt[:, uoff(p, g):uoff(p, g) + ua],
                             start=True, stop=True)
            nc.tensor.matmul(p0t[:, 512:512 + ub], a0[:],
                             zt[:, uoff(p, g + 1):uoff(p, g + 1) + ub],
                             start=True, stop=True)
            state[("p0", q)] = p0t

        def s_ev0(u):
            p, g = divmod(u, BLK)
            ub = ucols(p, g + 1)
            q = u // 2
            p0t = state.pop(("p0", q))
            y0t = y0p.tile([128, 1024], bf16, tag="y0", name="y0t")
            # one 1D evict across both banks (the inter-bank gap is junk)
            evict(q in ACT0, y0t[:, 0:512 + ub], p0t[:, 0:512 + ub], None)
            state[("y0", q)] = y0t

        def s_mm1(u):
            p, g = divmod(u, BLK)
            uc = ucols(p, g)
            q, half = divmod(u, 2)
            y0t = state[("y0", q)]
            if half == 1:
                del state[("y0", q)]
            p1t = p1p.tile([128, 512], f32, tag="p1", name="p1t")
            nc.tensor.matmul(p1t[:, 0:uc], w1[:],
                             y0t[:, half * 512:half * 512 + uc],
                             start=True, stop=True)
            state[("p1", u)] = p1t

        def s_ev1(u):
            p, g = divmod(u, BLK)
            pw = PHW[p]
            uc = ucols(p, g)
            p1t = state.pop(("p1", u))
            if g == 0:
                # per-phase y1 tile: slot 0 = the duplicated image row 0
                state[("y1ph", p)] = y1p.tile([128, (H + 1) * 28], bf16,
                                              tag="y1", name="y1t")
            y1t = state[("y1ph", p)]
            off = (RPB * g + (1 if g else 0)) * pw
            evict(u in ACT1, y1t[:, off:off + uc], p1t[:, 0:uc], b1ap)

        def s_mm2(u):
            p, g = divmod(u, BLK)
            pw = PHW[p]
            y1t = state[("y1ph", p)]
            if g == BLK - 1:
                del state[("y1ph", p)]
            if g == 0:
                state[("py2", p)] = py2p.tile([128, 512], f32, tag="py2",
                                              name="py2t")
            py2t = state[("py2", p)]
            # 18-row window: rows 17g-1 .. 17g+16 (slot 17g .. 17g+17)
            nc.tensor.matmul(py2t[:, 0:RPU * pw], w2g[g],
                             y1t[:, RPB * g * pw:(RPB * g + RPU) * pw],
                             start=(g == 0), stop=(g == BLK - 1),
                             skip_group_check=True)

        def s_ups1(p):
            pw = PHW[p]
            uc = RPU * pw
            py2t = state.pop(("py2", p))
            # sy[18, pw] = y2 logits (packed copy of py2)
            syt = syp.tile([128, 504], bf16, tag="sy", name="syt")
            sy3 = syt[:, 0:uc].rearrange("q (r c) -> q r c", r=RPU)
            if SY_ACT:
                nc.scalar.activation(syt[:, 0:uc], py2t[:, 0:uc], Act.Identity)
            else:
                nc.vector.tensor_copy(syt[:, 0:uc], py2t[:, 0:uc])
            state[("sy", p)] = syt
            if p >= 2:
                del state[("sy", p - 2)]
            # plane offsets in up/fo: [O_rc 17w][QQ 18w][O_er 17w][O_ee 17w]
            orc, oqq, oer, oee = (0, RPB * pw, (RPB + RPU) * pw,
                                  (2 * RPB + RPU) * pw)
            upt = upp.tile([128, max(PSTRIP)], bf16, tag="up", name="upt")
            state[("up", p)] = upt
            up_qq = upt[:, oqq:oer].rearrange("q (r c) -> q r c", r=RPU)
            # QQ[18,pw] = sy[., j-1] + sy[., j]; rows 1..17 are O_re.
            eng = nc.gpsimd if UPS_ENG[0] == "p" else nc.vector
            eng.tensor_tensor(up_qq[:, :, 1:pw],
                              sy3[:, :, 0:pw - 1], sy3[:, :, 1:pw], Alu.add)
            # QQ col 0 reads the previous phase's last col (phase 0 edge-pads)
            if p == 0:
                nc.gpsimd.tensor_tensor(up_qq[:, :, 0:1], sy3[:, :, 0:1],
                                        sy3[:, :, 0:1], Alu.add)
            else:
                ppw = PHW[p - 1]
                psy3 = state[("sy", p - 1)][:, 0:RPU * PHW[p - 1]].rearrange(
                    "q (r c) -> q r c", r=RPU)
                nc.gpsimd.tensor_tensor(up_qq[:, :, 0:1],
                                        psy3[:, :, ppw - 1:ppw],
                                        sy3[:, :, 0:1], Alu.add)
            # O_er[17,pw] = sy[k] + sy[k+1]
            eng = nc.gpsimd if UPS_ENG[1] == "p" else nc.vector
            eng.tensor_tensor(upt[:, oer:oee], syt[:, 0:RPB * pw],
                              syt[:, pw:uc], Alu.add)
            # O_rc[17,pw] = 2 * sy[1:18]
            eng = nc.gpsimd if UPS_ENG[3] == "p" else nc.vector
            eng.tensor_scalar(upt[:, orc:oqq], syt[:, pw:uc], 2.0, None,
                              Alu.mult)

        def s_ups2(p):
            pw = PHW[p]
            oqq, oer, oee, oend = (RPB * pw, (RPB + RPU) * pw,
                                   (2 * RPB + RPU) * pw, (3 * RPB + RPU) * pw)
            upt = state[("up", p)]
            # O_ee[17,pw] = QQ[k] + QQ[k+1] (4x the logit)
            eng = nc.gpsimd if UPS_ENG[2] == "p" else nc.vector
            eng.tensor_tensor(upt[:, oee:oend], upt[:, oqq:oqq + RPB * pw],
                              upt[:, oqq + pw:oer], Alu.add)
            # sigmoids over [O_rc] and [QQ|O_er] (all hold 2x the logit);
            # split so Act's in-order queue never blocks evictions for long
            fot = fop.tile([128, max(PSTRIP)], bf16, tag="fo", name="fot")
            nc.scalar.activation(fot[:, 0:oqq], upt[:, 0:oqq],
                                 Act.Sigmoid, bias=b2ap, scale=0.5)
            state[("fo", p)] = fot

        def s_ups2b(p):
            pw = PHW[p]
            oqq, oee = RPB * pw, (2 * RPB + RPU) * pw
            upt = state[("up", p)]
            fot = state[("fo", p)]
            nc.scalar.activation(fot[:, oqq:oee], upt[:, oqq:oee],
                                 Act.Sigmoid, bias=b2ap, scale=0.5)
            dst = outd[:, :, OOFF[p]:OOFF[p] + oee]
            nc.sync.dma_start(dst.rearrange("g i v -> (g i) v"),
                              fot[:, 0:oee])

        def s_ups3(p):
            pw = PHW[p]
            oee, oend = (2 * RPB + RPU) * pw, (3 * RPB + RPU) * pw
            upt = state.pop(("up", p))
            fot = state.pop(("fo", p))
            # O_ee holds 4x the logit
            nc.scalar.activation(fot[:, oee:oend], upt[:, oee:oend],
                                 Act.Sigmoid, bias=b2ap, scale=0.25)
            dst = outd[:, :, OOFF[p] + oee:OOFF[p + 1]]
            nc.sync.dma_start(dst.rearrange("g i v -> (g i) v"),
                              fot[:, oee:oend])

        ups_q = []  # [phase, mm2-done tick, next stage]
        TOTAL = N_UNITS + D_MM2 + U3 + 2
        for i in range(TOTAL):
            if i < N_UNITS and i % 2 == 0:
                s_mm0(i)
            j = i - D_EV0
            if 0 <= j < N_UNITS and j % 2 == 0:
                s_ev0(j)
            j = i - D_MM1
            if 0 <= j < N_UNITS:
                s_mm1(j)
            j = i - D_EV1
            if 0 <= j < N_UNITS:
                s_ev1(j)
            j = i - D_MM2
            if 0 <= j < N_UNITS:
                s_mm2(j)
                if j % BLK == BLK - 1:
                    ups_q.append([j // BLK, i, 1])
            last = i >= TOTAL - 1
            # once every matmul is emitted there is no PE pipeline left to
            # protect from Act head-of-line blocking: flush the remaining
            # upsample stages with tight spacing to shorten the drain tail
            drain = i >= N_UNITS + D_MM2 + int(os.environ.get("K_DREL", "99"))
            u2, u2b, u3 = ((2, 3, 4) if drain else (U2, U2 + 2, U3))
            for item in list(ups_q):
                p, t, st = item
                done = i - t
                if st == 1 and (done >= U1 or last):
                    s_ups1(p)
                    item[2] = st = 2
                if st == 2 and (done >= u2 or last):
                    s_ups2(p)
                    item[2] = st = 3
                if st == 3 and (done >= u2b or last):
                    s_ups2b(p)
                    item[2] = st = 4
                if st == 4 and (done >= u3 or last):
                    s_ups3(p)
                    ups_q.remove(item)

    nc.compile()
    return nc


def _host_prep(mask_feats, mask_head_params, locations, im_inds, fpn_levels,
               sizes_of_interest):
    import ml_dtypes
    bf16 = ml_dtypes.bfloat16

    mask_feats = np.asarray(mask_feats, dtype=np.float32)
    params = np.asarray(mask_head_params, dtype=np.float32)
    locations = np.asarray(locations, dtype=np.float32)
    im_inds = np.asarray(im_inds).astype(np.int64)
    soi_tab = np.asarray(sizes_of_interest, dtype=np.float32)
    fpn_levels = np.asarray(fpn_levels).astype(np.int64)

    w0 = params[:, 0:80].reshape(N_INST, CH, CIN + 2)
    w1 = params[:, 80:144].reshape(N_INST, CH, CH)
    w2 = params[:, 144:152].reshape(N_INST, 1, CH)
    b0 = params[:, 152:160]
    b1 = params[:, 160:168]
    b2 = params[:, 168:169]

    soi = soi_tab[fpn_levels]
    alpha = -w0[:, :, 0] / soi[:, None]
    beta = -w0[:, :, 1] / soi[:, None]
    c0 = b0 + (w0[:, :, 0] * locations[:, 0:1]
               + w0[:, :, 1] * locations[:, 1:2]) / soi[:, None]
    wfeat = w0[:, :, 2:]

    stride = 8
    xs = np.arange(W, dtype=np.float32) * stride + stride // 2
    ys = np.arange(H, dtype=np.float32) * stride + stride // 2
    z3 = np.empty((K0, H, W), np.float32)
    z3[0] = xs[None, :]
    z3[1] = ys[:, None]
    z3[2] = 1.0
    z3[3:] = mask_feats.reshape(N_IMG * CIN, H, W)
    # phase-major strips of 17-row blocks; each strip leads with a
    # duplicate of image row 0 (block 0's edge-pad halo row)
    zb = z3[:, np.concatenate([[0], np.arange(H)]), :]        # (K0, 137, W)
    coff = np.cumsum([0] + PHW).tolist()
    strips = []
    for p in range(NPH):
        s = zb[:, :, coff[p]:coff[p + 1]]                     # (K0, 137, w)
        strips.append(s.reshape(K0, (H + 1) * PHW[p]))
    z = np.ascontiguousarray(np.concatenate(strips, axis=1)).astype(bf16)

    in_maps = []
    for c in range(N_CORES):
        a0 = np.zeros((K0, 128), np.float32)
        w1p = np.zeros((128, 128), np.float32)
        w2p = np.zeros((128, 368), np.float32)
        bbv = np.zeros((128, 2), np.float32)
        for i in range(IPC):
            gi = IPC * c + i
            for o in range(CH):
                m = CH * i + o
                a0[0, m] = alpha[gi, o]
                a0[1, m] = beta[gi, o]
                a0[2, m] = c0[gi, o]
                base = 3 + CIN * int(im_inds[gi])
                a0[base:base + CIN, m] = wfeat[gi, o, :]
                w1p[CH * i:CH * i + CH, m] = w1[gi, o, :]
                bbv[m, 0] = b1[gi, o]
            w2p[CH * i:CH * i + CH, 240 + i] = w2[gi, 0, :]
        for q in range(128):
            bbv[q, 1] = b2[IPC * c + (q % IPC), 0]
        in_maps.append({
            "z_in": z,
            "a0t_in": a0.astype(bf16),
            "w1_in": w1p.astype(bf16),
            "w2_in": w2p.astype(bf16),
            "b_in": bbv,
        })
    return in_maps


def kernel(mask_feats, mask_head_params, locations, im_inds, fpn_levels,
           sizes_of_interest, mask_feat_stride):
    global LAST_EXEC_TIME_NS
    assert int(mask_feat_stride) == 8, "kernel hardcodes mask_feat_stride=8"

    os.environ["BASS_NEVER_TRACE"] = "1"
    from concourse.bass_utils import run_bass_kernel_spmd

    in_maps = _host_prep(mask_feats, mask_head_params, locations, im_inds,
                         fpn_levels, sizes_of_interest)

    if "nc" not in _CACHE:
        _CACHE["nc"] = _build_program()
    nc = _CACHE["nc"]

    res = run_bass_kernel_spmd(nc, in_maps, list(range(N_CORES)), trace=False)
    LAST_EXEC_TIME_NS = res.exec_time_ns

    coff = np.cumsum([0] + PHW).tolist()
    out = np.empty((N_INST, 1, OH, OW), np.float32)
    for c in range(N_CORES):
        dev = np.asarray(res.results[c]["out"]).astype(np.float32)
        # dev: [blk g, inst i, strip cols]
        o6 = np.empty((IPC, BLK, RPB, 2, OW), np.float32)
        for p in range(NPH):
            pw = PHW[p]
            orc, oqq, oer, oee = (np.array([0, RPB, RPB + RPU, 2 * RPB + RPU])
                                  * pw + OOFF[p])
            def plane(off, r):
                pl = dev[:, :, off:off + r * pw].reshape(BLK, IPC, r, pw)
                return pl.transpose(1, 0, 2, 3)
            c0_, c1_ = 2 * coff[p], 2 * coff[p + 1]
            o6[:, :, :, 1, c0_ + 1:c1_:2] = plane(orc, RPB)
            o6[:, :, :, 1, c0_:c1_:2] = plane(oqq, RPU)[:, :, 1:, :]
            o6[:, :, :, 0, c0_ + 1:c1_:2] = plane(oer, RPB)
            o6[:, :, :, 0, c0_:c1_:2] = plane(oee, RPB)
        out[IPC * c:IPC * (c + 1), 0] = o6.transpose(0, 1, 2, 3, 4).reshape(
            IPC, OH, OW)
    return out


# revision 28
# speedup vs baseline: 1.1637x; 1.0561x over previous
"""
Trainium2 Bass kernel for CondConv mask head (CondInst-style dynamic mask head).

Computation (fixed problem size):
  mask_feats (2, 8, 136, 200), 128 instances with per-instance 169 params
  -> per-instance 3-layer 1x1 convs over [rel_coords(2); feats(8)] -> (128,1,136,200)
  -> aligned_bilinear x2 upsample -> sigmoid -> (128, 1, 272, 400)

Strategy (8 NeuronCores, 16 instances per core), v2:
  * All matmul operands are bf16 (1 PE cycle/col).  Host folds rel-coords
    into a shared 19-row spatial matrix Z = [x; y; 1; feats_im0; feats_im1]
    and per-core lhsTs (a0 with the c0 constant on the ones-row,
    block-diagonal w1, and a zero-padded w2 strip whose eight 128-wide
    windows place the w2 block at lhsT columns 16g).
  * The image is processed in 8 column-phases of tunable widths (default
    20,28x6,12 -- narrow last phase to shorten the drain tail); each phase
    is 8 row-block units.  Row blocks OVERLAP by one row (18 rows, width*18
    cols <= 504): block g covers image rows 17g-1 .. 17g+16 (block 0
    edge-pads by duplicating row 0 in Z), so the x2-upsample top halo row
    is computed locally and no cross-partition halo DMA is needed.
  * mm0 writes unit PAIRS into a 2-bank PSUM tile (outs at col 0 and 512 so
    each matmul stays within a bank); the pair is evicted with one relu
    instruction (2D access pattern).  mm1/evict are per-unit; mm2
    accumulates phase logits into py2[128 = 8 blk x 16 inst].  PSUM-reading
    evictions run on ScalarE/VectorE only (GPSIMD has no PSUM port); the
    Act/DVE split is schedule-tuned.  PSUM: p0 2x2 banks + p1 2 + py2 2 = 8.
  * Upsample = 4 polyphase planes, packed bf16 in SBUF (VectorE 2x/4x
    modes, GpSimd helps off the critical chain):
      sy   = py2 copy (packed [18 x W'])
      O_rc = 2*sy[1:18]                 (odd row, odd col)
      QQ   = sy[.,j-1]+sy[.,j] (18 rows; rows 1..17 are the O_re plane;
             col 0 reads the previous phase's last col, tiny separate op)
      O_er = sy[k]+sy[k+1]              (even row, odd col)
      O_ee = QQ[k]+QQ[k+1]              (even row, even col, 4x logit)
  * Sigmoids: one Act instruction over [O_rc|QQ|O_er] (all 2x the logit:
    scale 0.5, bias b2) and one over O_ee (scale 0.25), emitted several
    units later so Act's in-order queue never head-blocks on the planes ->
    contiguous bf16 fo tile -> one 128-descriptor DMA per phase into
    [blk, inst, phase-strip] DRAM.  Host interleaves the planes, f32-casts.
"""

import os
import numpy as np

CH = 8
CIN = 8
N_IMG, H, W = 2, 136, 200
HW = H * W
N_INST = 128
N_CORES = 8
IPC = 16                         # instances per core
FACTOR = 2
OH, OW = H * FACTOR, W * FACTOR  # 272, 400
BLK = 8                          # row-blocks (= units) per phase
RPB = H // BLK                   # 17 output rows per block
RPU = RPB + 1                    # 18 stored rows per unit (one overlap row)
K0 = 3 + N_IMG * CIN             # 19 contraction rows for layer 0

PHW = [int(x) for x in os.environ.get(
    "K_PHW", "12,28,28,28,28,28,28,20").split(",")]
NPH = len(PHW)
assert sum(PHW) == W and all(w <= 28 for w in PHW)
# processing order: rotate so a narrow phase is processed LAST (short drain
# tail).  The first-processed phase carries one extra leading z column (its
# left overlap); every other phase reads its left neighbour's sy, which the
# rotation guarantees was processed just before (spatial phase 0 edge-pads).
ROT = int(os.environ.get("K_ROT", "0")) % NPH
PORD = [(ROT + k) % NPH for k in range(NPH)]
PPW = [PHW[p] for p in PORD]              # width per processing position
POV = [1 if (k == 0 and PORD[0] != 0) else 0 for k in range(NPH)]
PWE = [w + o for w, o in zip(PPW, POV)]   # effective (stored) width
assert all(w <= 28 for w in PWE)
# unit 0 of each phase has 18 rows (leading duplicate of image row 0, the
# edge-pad halo for block 0's 18-row mm2 window)
ZOFF = np.cumsum([0] + [(H + 1) * w for w in PWE]).tolist()
ZHW = ZOFF[-1]
PSTRIP = [(3 * RPB + RPU) * w for w in PPW]   # 69*w out cols per position
OOFF = np.cumsum([0] + PSTRIP).tolist()
N_UNITS = NPH * BLK

N_WARM = int(os.environ.get("K_WARM", "3"))
EV0A = int(os.environ.get("K_EV0A", "29"))   # of 32 pair evicts (y0) on Act
EV1A = int(os.environ.get("K_EV1A", "0"))   # of 64 single evicts (y1) on Act

LAST_EXEC_TIME_NS = None
_CACHE = {}


def _spread(n_act, total, skip=int(os.environ.get("K_SKIP0", "1"))):
    # spread n_act picks over [skip, total) so the first evictions (pipeline
    # fill, before Act's activation tables are loaded) go to DVE
    if n_act <= 0:
        return set()
    n_act = min(n_act, total - skip)
    return set((skip + np.arange(n_act) * (total - skip) // n_act).tolist())


def _build_program():
    import concourse.bass as bass
    import concourse.bacc as bacc
    import concourse.tile as tile
    from concourse import mybir
    from contextlib import ExitStack

    f32 = mybir.dt.float32
    bf16 = mybir.dt.bfloat16
    Alu = mybir.AluOpType
    Act = mybir.ActivationFunctionType

    nc = bacc.Bacc("TRN2", target_bir_lowering=False, debug=False)

    zd = nc.dram_tensor("z_in", [K0, ZHW], bf16, kind="ExternalInput").ap()
    a0d = nc.dram_tensor("a0t_in", [K0, 128], bf16, kind="ExternalInput").ap()
    w1d = nc.dram_tensor("w1_in", [128, 128], bf16, kind="ExternalInput").ap()
    w2d = nc.dram_tensor("w2_in", [128, 368], bf16, kind="ExternalInput").ap()
    bd = nc.dram_tensor("b_in", [128, 2], f32, kind="ExternalInput").ap()
    outd = nc.dram_tensor("out", [BLK, IPC, OOFF[-1]], bf16,
                          kind="ExternalOutput").ap()

    EV1F = int(os.environ.get("K_EV1F", "0"))
    ACT0 = _spread(EV0A, N_UNITS // 2)
    ACT1 = _spread(EV1A, N_UNITS, skip=5) | set(range(2, 2 + EV1F))

    U1 = int(os.environ.get("K_U1", "1"))
    U2 = int(os.environ.get("K_U2", "11"))
    U3 = int(os.environ.get("K_U3", "14"))
    D_EV0 = int(os.environ.get("K_DEV0", "1"))
    D_MM1 = int(os.environ.get("K_DMM1", "3"))
    D_EV1 = int(os.environ.get("K_DEV1", "5"))
    D_MM2 = int(os.environ.get("K_DMM2", "7"))
    # engines for [QQ, O_er, O_ee, O_rc]: p=pool, d=dve
    UPS_ENG = os.environ.get("K_UPS", "pppd")
    SY_ACT = os.environ.get("K_SY", "act") == "act"
    SIG_MERGE = os.environ.get("K_SIGM", "0") == "1"

    with tile.TileContext(nc) as tc, ExitStack() as ctx:
        consts = ctx.enter_context(tc.tile_pool(name="consts", bufs=1))
        zt = consts.tile([K0, ZHW], bf16)
        a0 = consts.tile([K0, 128], bf16)
        w1 = consts.tile([128, 128], bf16)
        w2s = consts.tile([128, 368], bf16)
        bb = consts.tile([128, 2], f32)
        warm = consts.tile([K0, 512], bf16)
        scr = consts.tile([1, 8], f32)

        # activation-table preloads first: tiny memset, then one dummy
        # activation per function so the table loads absorb into the idle
        # start instead of blocking the first Act evictions
        nc.vector.memset(scr[:], 0.0)
        nc.scalar.activation(scr[:], scr[:], Act.Sigmoid)
        nc.scalar.activation(scr[:], scr[:], Act.Relu)
        nc.scalar.activation(scr[:], scr[:], Act.Identity)
        nc.vector.memset(warm[:], 0.0)

        # z for the first unit-pair first (mm0 starts ASAP), then consts,
        # then the rest of z
        nc.sync.dma_start(zt[:, 0:ZOFF[1]], zd[:, 0:ZOFF[1]])
        nc.sync.dma_start(a0[:], a0d)
        nc.sync.dma_start(bb[:], bd)
        nc.sync.dma_start(w1[:], w1d)
        nc.sync.dma_start(w2s[:], w2d)
        nc.sync.dma_start(zt[:, ZOFF[1]:ZOFF[4]], zd[:, ZOFF[1]:ZOFF[4]])
        nc.sync.dma_start(zt[:, ZOFF[4]:], zd[:, ZOFF[4]:])

        b1ap = bb[:, 0:1]
        b2ap = bb[:, 1:2]
        w2g = [w2s[:, 240 - 16 * g:368 - 16 * g] for g in range(BLK)]

        p0p = ctx.enter_context(tc.tile_pool(name="p0p", bufs=2, space="PSUM"))
        p1p = ctx.enter_context(tc.tile_pool(name="p1p", bufs=2, space="PSUM"))
        py2p = ctx.enter_context(tc.tile_pool(name="py2p", bufs=2, space="PSUM"))
        y0p = ctx.enter_context(tc.tile_pool(name="y0p", bufs=3))
        y1p = ctx.enter_context(tc.tile_pool(name="y1p", bufs=2))
        syp = ctx.enter_context(tc.tile_pool(name="syp", bufs=3))
        upp = ctx.enter_context(tc.tile_pool(name="upp", bufs=3))
        fop = ctx.enter_context(tc.tile_pool(name="fop", bufs=3))

        # PE warm-up while the first z chunk's DMA is in flight
        for wi in range(N_WARM):
            pw = p0p.tile([128, 1024], f32, tag="p0", name="pw")
            nc.tensor.matmul(pw[:, 0:512], warm[:, 0:128], warm[:],
                             start=True, stop=True)

        state = {}

        def evict(on_act, dst, src, bias_ap):
            if on_act:
                if bias_ap is None:
                    nc.scalar.activation(dst, src, Act.Relu)
                else:
                    nc.scalar.activation(dst, src, Act.Relu, bias=bias_ap)
            else:
                if bias_ap is None:
                    nc.vector.tensor_scalar(dst, src, 0.0, None, Alu.max)
                else:
                    nc.vector.tensor_scalar(dst, src, bias_ap, 0.0,
                                            Alu.add, Alu.max)

        def ucols(p, g):
            # unit 0 of a phase carries the extra duplicate row (18 rows)
            return (RPB + (1 if g == 0 else 0)) * PWE[p]

        def uoff(p, g):
            return ZOFF[p] + (RPB * g + (1 if g else 0)) * PWE[p]

        def s_mm0(u):
            p, g = divmod(u, BLK)
            q = u // 2
            ua, ub = ucols(p, g), ucols(p, g + 1)
            p0t = p0p.tile([128, 1024], f32, tag="p0", name="p0t")
            nc.tensor.matmul(p0t[:, 0:ua], a0[:],
                             zt[:, uoff(p, g):uoff(p, g) + ua],
                             start=True, stop=True)
            nc.tensor.matmul(p0t[:, 512:512 + ub], a0[:],
                             zt[:, uoff(p, g + 1):uoff(p, g + 1) + ub],
                             start=True, stop=True)
            state[("p0", q)] = p0t

        def s_ev0(u):
            p, g = divmod(u, BLK)
            ua, ub = ucols(p, g), ucols(p, g + 1)
            q = u // 2
            p0t = state.pop(("p0", q))
            y0t = y0p.tile([128, 1024], bf16, tag="y0", name="y0t")
            if ua == ub:
                src2 = p0t[:].rearrange("p (b c) -> p b c", b=2)[:, :, 0:ua]
                dst2 = y0t[:].rearrange("p (b c) -> p b c", b=2)[:, :, 0:ua]
                evict(q in ACT0, dst2, src2, None)
            else:
                # unequal halves (18-row unit 0): one 1D evict across both
                # banks, the inter-bank gap is junk
                evict(q in ACT0, y0t[:, 0:512 + ub], p0t[:, 0:512 + ub], None)
            state[("y0", q)] = y0t

        def s_mm1(u):
            p, g = divmod(u, BLK)
            uc = ucols(p, g)
            q, half = divmod(u, 2)
            y0t = state[("y0", q)]
            if half == 1:
                del state[("y0", q)]
            p1t = p1p.tile([128, 512], f32, tag="p1", name="p1t")
            nc.tensor.matmul(p1t[:, 0:uc], w1[:],
                             y0t[:, half * 512:half * 512 + uc],
                             start=True, stop=True)
            state[("p1", u)] = p1t

        def s_ev1(u):
            p, g = divmod(u, BLK)
            pw = PWE[p]
            uc = ucols(p, g)
            p1t = state.pop(("p1", u))
            if g == 0:
                # per-phase y1 tile: slot 0 = the duplicated image row 0
                state[("y1ph", p)] = y1p.tile([128, (H + 1) * 28], bf16,
                                              tag="y1", name="y1t")
            y1t = state[("y1ph", p)]
            off = (RPB * g + (1 if g else 0)) * pw
            evict(u in ACT1, y1t[:, off:off + uc], p1t[:, 0:uc], b1ap)

        def s_mm2(u):
            p, g = divmod(u, BLK)
            pw = PWE[p]
            y1t = state[("y1ph", p)]
            if g == BLK - 1:
                del state[("y1ph", p)]
            if g == 0:
                state[("py2", p)] = py2p.tile([128, 512], f32, tag="py2",
                                              name="py2t")
            py2t = state[("py2", p)]
            # 18-row window: rows 17g-1 .. 17g+16 (slot 17g .. 17g+17)
            nc.tensor.matmul(py2t[:, 0:RPU * pw], w2g[g],
                             y1t[:, RPB * g * pw:(RPB * g + RPU) * pw],
                             start=(g == 0), stop=(g == BLK - 1),
                             skip_group_check=True)

        def s_ups1(p):
            pw = PPW[p]
            ov = POV[p]
            we = PWE[p]
            uc = RPU * we
            sp = PORD[p]                  # spatial phase index
            py2t = state.pop(("py2", p))
            # sy[18, we] = y2 logits (packed copy of py2)
            syt = syp.tile([128, 504], bf16, tag="sy", name="syt")
            sy3 = syt[:, 0:uc].rearrange("q (r c) -> q r c", r=RPU)
            if SY_ACT:
                nc.scalar.activation(syt[:, 0:uc], py2t[:, 0:uc], Act.Identity)
            else:
                nc.vector.tensor_copy(syt[:, 0:uc], py2t[:, 0:uc])
            state[("sy", p)] = syt
            if p >= 2:
                del state[("sy", p - 2)]
            # plane offsets in up/fo: [O_rc 17w][QQ 18w][O_er 17w][O_ee 17w]
            orc, oqq, oer, oee = (0, RPB * pw, (RPB + RPU) * pw,
                                  (2 * RPB + RPU) * pw)
            upt = upp.tile([128, max(PSTRIP)], bf16, tag="up", name="upt")
            state[("up", p)] = upt
            up_qq = upt[:, oqq:oer].rearrange("q (r c) -> q r c", r=RPU)
            lastp = p == NPH - 1
            # QQ[18,pw] = y2[., J-1] + y2[., J]; rows 1..17 are O_re.
            eng = nc.gpsimd if UPS_ENG[0] == "p" and not lastp else nc.vector
            if ov:
                # own leading overlap column: one full-width op
                eng.tensor_tensor(up_qq[:], sy3[:, :, 0:pw],
                                  sy3[:, :, 1:we], Alu.add)
            else:
                eng.tensor_tensor(up_qq[:, :, 1:pw], sy3[:, :, 0:pw - 1],
                                  sy3[:, :, 1:pw], Alu.add)
                if sp == 0:
                    # left image edge: y2[-1] := y2[0]
                    nc.gpsimd.tensor_tensor(up_qq[:, :, 0:1], sy3[:, :, 0:1],
                                            sy3[:, :, 0:1], Alu.add)
                else:
                    pwe = PWE[p - 1]
                    psy3 = state[("sy", p - 1)][:, 0:RPU * pwe].rearrange(
                        "q (r c) -> q r c", r=RPU)
                    nc.gpsimd.tensor_tensor(up_qq[:, :, 0:1],
                                            psy3[:, :, pwe - 1:pwe],
                                            sy3[:, :, 0:1], Alu.add)
            # O_er[17,pw] = y2[k] + y2[k+1]
            eng = nc.gpsimd if UPS_ENG[1] == "p" and not lastp else nc.vector
            if ov:
                eng.tensor_tensor(
                    upt[:, oer:oee].rearrange("q (r c) -> q r c", r=RPB),
                    sy3[:, 0:RPB, 1:we], sy3[:, 1:RPU, 1:we], Alu.add)
            else:
                eng.tensor_tensor(upt[:, oer:oee], syt[:, 0:RPB * pw],
                                  syt[:, pw:uc], Alu.add)
            # O_rc[17,pw] = 2 * y2[1:18]
            eng = nc.gpsimd if UPS_ENG[3] == "p" else nc.vector
            if ov:
                eng.tensor_scalar(
                    upt[:, orc:oqq].rearrange("q (r c) -> q r c", r=RPB),
                    sy3[:, 1:RPU, 1:we], 2.0, None, Alu.mult)
            else:
                eng.tensor_scalar(upt[:, orc:oqq], syt[:, pw:uc], 2.0, None,
                                  Alu.mult)

        def s_ups2(p):
            pw = PPW[p]
            oqq, oer, oee, oend = (RPB * pw, (RPB + RPU) * pw,
                                   (2 * RPB + RPU) * pw, (3 * RPB + RPU) * pw)
            upt = state[("up", p)]
            # O_ee[17,pw] = QQ[k] + QQ[k+1] (4x the logit)
            eng = (nc.gpsimd if UPS_ENG[2] == "p" and p != NPH - 1
                   else nc.vector)
            eng.tensor_tensor(upt[:, oee:oend], upt[:, oqq:oqq + RPB * pw],
                              upt[:, oqq + pw:oer], Alu.add)
            # sigmoids over [O_rc] and [QQ|O_er] (all hold 2x the logit);
            # split so Act's in-order queue never blocks evictions for long
            fot = fop.tile([128, max(PSTRIP)], bf16, tag="fo", name="fot")
            if SIG_MERGE:
                nc.scalar.activation(fot[:, 0:oee], upt[:, 0:oee],
                                     Act.Sigmoid, bias=b2ap, scale=0.5)
            else:
                nc.scalar.activation(fot[:, 0:oqq], upt[:, 0:oqq],
                                     Act.Sigmoid, bias=b2ap, scale=0.5)
            state[("fo", p)] = fot

        def s_ups2b(p):
            pw = PPW[p]
            oqq, oee = RPB * pw, (2 * RPB + RPU) * pw
            upt = state[("up", p)]
            fot = state[("fo", p)]
            if not SIG_MERGE:
                nc.scalar.activation(fot[:, oqq:oee], upt[:, oqq:oee],
                                     Act.Sigmoid, bias=b2ap, scale=0.5)
            dst = outd[:, :, OOFF[p]:OOFF[p] + oee]
            nc.sync.dma_start(dst.rearrange("g i v -> (g i) v"),
                              fot[:, 0:oee])

        def s_ups3(p):
            pw = PPW[p]
            oee, oend = (2 * RPB + RPU) * pw, (3 * RPB + RPU) * pw
            upt = state.pop(("up", p))
            fot = state.pop(("fo", p))
            # O_ee holds 4x the logit
            nc.scalar.activation(fot[:, oee:oend], upt[:, oee:oend],
                                 Act.Sigmoid, bias=b2ap, scale=0.25)
            dst = outd[:, :, OOFF[p] + oee:OOFF[p + 1]]
            nc.sync.dma_start(dst.rearrange("g i v -> (g i) v"),
                              fot[:, oee:oend])

        ups_q = []  # [phase, mm2-done tick, next stage]
        TOTAL = N_UNITS + D_MM2 + U3 + 2
        for i in range(TOTAL):
            if i < N_UNITS and i % 2 == 0:
                s_mm0(i)
            j = i - D_EV0
            if 0 <= j < N_UNITS and j % 2 == 0:
                s_ev0(j)
            j = i - D_MM1
            if 0 <= j < N_UNITS:
                s_mm1(j)
            j = i - D_EV1
            if 0 <= j < N_UNITS:
                s_ev1(j)
            j = i - D_MM2
            if 0 <= j < N_UNITS:
                s_mm2(j)
                if j % BLK == BLK - 1:
                    ups_q.append([j // BLK, i, 1])
            last = i >= TOTAL - 1
            # once every matmul is emitted there is no PE pipeline left to
            # protect from Act head-of-line blocking: flush the remaining
            # upsample stages with tight spacing to shorten the drain tail
            drain = i >= N_UNITS + D_MM2 + int(os.environ.get("K_DREL", "99"))
            UTAIL = int(os.environ.get("K_UTAIL", "99"))
            for item in list(ups_q):
                p, t, st = item
                done = i - t
                if drain or p >= NPH - 2:
                    u2, u2b, u3 = ((2, 3, 4) if drain
                                   else (min(U2, UTAIL), min(U2 + 2, UTAIL + 2),
                                         min(U3, UTAIL + 4)))
                else:
                    u2, u2b, u3 = U2, U2 + 2, U3
                if st == 1 and (done >= U1 or last):
                    s_ups1(p)
                    item[2] = st = 2
                if st == 2 and (done >= u2 or last):
                    s_ups2(p)
                    item[2] = st = 3
                if st == 3 and (done >= u2b or last):
                    s_ups2b(p)
                    item[2] = st = 4
                if st == 4 and (done >= u3 or last):
                    s_ups3(p)
                    ups_q.remove(item)

    nc.compile()
    return nc


def _host_prep(mask_feats, mask_head_params, locations, im_inds, fpn_levels,
               sizes_of_interest):
    import ml_dtypes
    bf16 = ml_dtypes.bfloat16

    mask_feats = np.asarray(mask_feats, dtype=np.float32)
    params = np.asarray(mask_head_params, dtype=np.float32)
    locations = np.asarray(locations, dtype=np.float32)
    im_inds = np.asarray(im_inds).astype(np.int64)
    soi_tab = np.asarray(sizes_of_interest, dtype=np.float32)
    fpn_levels = np.asarray(fpn_levels).astype(np.int64)

    w0 = params[:, 0:80].reshape(N_INST, CH, CIN + 2)
    w1 = params[:, 80:144].reshape(N_INST, CH, CH)
    w2 = params[:, 144:152].reshape(N_INST, 1, CH)
    b0 = params[:, 152:160]
    b1 = params[:, 160:168]
    b2 = params[:, 168:169]

    soi = soi_tab[fpn_levels]
    alpha = -w0[:, :, 0] / soi[:, None]
    beta = -w0[:, :, 1] / soi[:, None]
    c0 = b0 + (w0[:, :, 0] * locations[:, 0:1]
               + w0[:, :, 1] * locations[:, 1:2]) / soi[:, None]
    wfeat = w0[:, :, 2:]

    stride = 8
    xs = np.arange(W, dtype=np.float32) * stride + stride // 2
    ys = np.arange(H, dtype=np.float32) * stride + stride // 2
    z3 = np.empty((K0, H, W), np.float32)
    z3[0] = xs[None, :]
    z3[1] = ys[:, None]
    z3[2] = 1.0
    z3[3:] = mask_feats.reshape(N_IMG * CIN, H, W)
    # strips in PROCESSING order; each strip leads with a duplicate of
    # image row 0 (block 0's edge-pad halo row); the first-processed strip
    # also carries its left-overlap column
    zb = z3[:, np.concatenate([[0], np.arange(H)]), :]        # (K0, 137, W)
    coff = np.cumsum([0] + PHW).tolist()
    strips = []
    for k in range(NPH):
        p = PORD[k]
        s = zb[:, :, coff[p] - POV[k]:coff[p + 1]]            # (K0, 137, we)
        strips.append(s.reshape(K0, (H + 1) * PWE[k]))
    z = np.ascontiguousarray(np.concatenate(strips, axis=1)).astype(bf16)

    in_maps = []
    for c in range(N_CORES):
        a0 = np.zeros((K0, 128), np.float32)
        w1p = np.zeros((128, 128), np.float32)
        w2p = np.zeros((128, 368), np.float32)
        bbv = np.zeros((128, 2), np.float32)
        for i in range(IPC):
            gi = IPC * c + i
            for o in range(CH):
                m = CH * i + o
                a0[0, m] = alpha[gi, o]
                a0[1, m] = beta[gi, o]
                a0[2, m] = c0[gi, o]
                base = 3 + CIN * int(im_inds[gi])
                a0[base:base + CIN, m] = wfeat[gi, o, :]
                w1p[CH * i:CH * i + CH, m] = w1[gi, o, :]
                bbv[m, 0] = b1[gi, o]
            w2p[CH * i:CH * i + CH, 240 + i] = w2[gi, 0, :]
        for q in range(128):
            bbv[q, 1] = b2[IPC * c + (q % IPC), 0]
        in_maps.append({
            "z_in": z,
            "a0t_in": a0.astype(bf16),
            "w1_in": w1p.astype(bf16),
            "w2_in": w2p.astype(bf16),
            "b_in": bbv,
        })
    return in_maps


def kernel(mask_feats, mask_head_params, locations, im_inds, fpn_levels,
           sizes_of_interest, mask_feat_stride):
    global LAST_EXEC_TIME_NS
    assert int(mask_feat_stride) == 8, "kernel hardcodes mask_feat_stride=8"

    os.environ["BASS_NEVER_TRACE"] = "1"
    from concourse.bass_utils import run_bass_kernel_spmd

    in_maps = _host_prep(mask_feats, mask_head_params, locations, im_inds,
                         fpn_levels, sizes_of_interest)

    if "nc" not in _CACHE:
        _CACHE["nc"] = _build_program()
    nc = _CACHE["nc"]

    res = run_bass_kernel_spmd(nc, in_maps, list(range(N_CORES)), trace=False)
    LAST_EXEC_TIME_NS = res.exec_time_ns

    coff = np.cumsum([0] + PHW).tolist()
    out = np.empty((N_INST, 1, OH, OW), np.float32)
    for c in range(N_CORES):
        dev = np.asarray(res.results[c]["out"]).astype(np.float32)
        # dev: [blk g, inst i, strip cols]
        o6 = np.empty((IPC, BLK, RPB, 2, OW), np.float32)
        for k in range(NPH):
            p = PORD[k]
            pw = PHW[p]
            orc, oqq, oer, oee = (np.array([0, RPB, RPB + RPU, 2 * RPB + RPU])
                                  * pw + OOFF[k])
            def plane(off, r):
                pl = dev[:, :, off:off + r * pw].reshape(BLK, IPC, r, pw)
                return pl.transpose(1, 0, 2, 3)
            c0_, c1_ = 2 * coff[p], 2 * coff[p + 1]
            o6[:, :, :, 1, c0_ + 1:c1_:2] = plane(orc, RPB)
            o6[:, :, :, 1, c0_:c1_:2] = plane(oqq, RPU)[:, :, 1:, :]
            o6[:, :, :, 0, c0_ + 1:c1_:2] = plane(oer, RPB)
            o6[:, :, :, 0, c0_:c1_:2] = plane(oee, RPB)
        out[IPC * c:IPC * (c + 1), 0] = o6.transpose(0, 1, 2, 3, 4).reshape(
            IPC, OH, OW)
    return out


# revision 30
# speedup vs baseline: 1.1902x; 1.0228x over previous
"""
Trainium2 Bass kernel for CondConv mask head (CondInst-style dynamic mask head).

Computation (fixed problem size):
  mask_feats (2, 8, 136, 200), 128 instances with per-instance 169 params
  -> per-instance 3-layer 1x1 convs over [rel_coords(2); feats(8)] -> (128,1,136,200)
  -> aligned_bilinear x2 upsample -> sigmoid -> (128, 1, 272, 400)

Strategy (8 NeuronCores, 16 instances per core), v2:
  * All matmul operands are bf16 (1 PE cycle/col).  Host folds rel-coords
    into a shared 19-row spatial matrix Z = [x; y; 1; feats_im0; feats_im1]
    and per-core lhsTs (a0 with the c0 constant on the ones-row,
    block-diagonal w1, and a zero-padded w2 strip whose eight 128-wide
    windows place the w2 block at lhsT columns 16g).
  * The image is processed in 8 column-phases of tunable widths (default
    20,28x6,12 -- narrow last phase to shorten the drain tail); each phase
    is 8 row-block units.  Row blocks OVERLAP by one row (18 rows, width*18
    cols <= 504): block g covers image rows 17g-1 .. 17g+16 (block 0
    edge-pads by duplicating row 0 in Z), so the x2-upsample top halo row
    is computed locally and no cross-partition halo DMA is needed.
  * mm0 writes unit PAIRS into a 2-bank PSUM tile (outs at col 0 and 512 so
    each matmul stays within a bank); the pair is evicted with one relu
    instruction (2D access pattern).  mm1/evict are per-unit; mm2
    accumulates phase logits into py2[128 = 8 blk x 16 inst].  PSUM-reading
    evictions run on ScalarE/VectorE only (GPSIMD has no PSUM port); the
    Act/DVE split is schedule-tuned.  PSUM: p0 2x2 banks + p1 2 + py2 2 = 8.
  * Upsample = 4 polyphase planes, packed bf16 in SBUF (VectorE 2x/4x
    modes, GpSimd helps off the critical chain):
      sy   = py2 copy (packed [18 x W'])
      O_rc = 2*sy[1:18]                 (odd row, odd col)
      QQ   = sy[.,j-1]+sy[.,j] (18 rows; rows 1..17 are the O_re plane;
             col 0 reads the previous phase's last col, tiny separate op)
      O_er = sy[k]+sy[k+1]              (even row, odd col)
      O_ee = QQ[k]+QQ[k+1]              (even row, even col, 4x logit)
  * Sigmoids: one Act instruction over [O_rc|QQ|O_er] (all 2x the logit:
    scale 0.5, bias b2) and one over O_ee (scale 0.25), emitted several
    units later so Act's in-order queue never head-blocks on the planes ->
    contiguous bf16 fo tile -> one 128-descriptor DMA per phase into
    [blk, inst, phase-strip] DRAM.  Host interleaves the planes, f32-casts.
"""

import os
import numpy as np

CH = 8
CIN = 8
N_IMG, H, W = 2, 136, 200
HW = H * W
N_INST = 128
N_CORES = 8
IPC = 16                         # instances per core
FACTOR = 2
OH, OW = H * FACTOR, W * FACTOR  # 272, 400
BLK = 8                          # row-blocks (= units) per phase
RPB = H // BLK                   # 17 output rows per block
RPU = RPB + 1                    # 18 stored rows per unit (one overlap row)
K0 = 3 + N_IMG * CIN             # 19 contraction rows for layer 0

PHW = [int(x) for x in os.environ.get(
    "K_PHW", "6,26,28,28,28,28,28,28").split(",")]
NPH = len(PHW)
assert sum(PHW) == W and all(w <= 28 for w in PHW)
# processing order: rotate so a narrow phase is processed LAST (short drain
# tail).  The first-processed phase carries one extra leading z column (its
# left overlap); every other phase reads its left neighbour's sy, which the
# rotation guarantees was processed just before (spatial phase 0 edge-pads).
ROT = int(os.environ.get("K_ROT", "0")) % NPH
PORD = [(ROT + k) % NPH for k in range(NPH)]
PPW = [PHW[p] for p in PORD]              # width per processing position
POV = [1 if (k == 0 and PORD[0] != 0) else 0 for k in range(NPH)]
PWE = [w + o for w, o in zip(PPW, POV)]   # effective (stored) width
assert all(w <= 28 for w in PWE)
# unit 0 of each phase has 18 rows (leading duplicate of image row 0, the
# edge-pad halo for block 0's 18-row mm2 window)
ZOFF = np.cumsum([0] + [(H + 1) * w for w in PWE]).tolist()
ZHW = ZOFF[-1]
PSTRIP = [(3 * RPB + RPU) * w for w in PPW]   # 69*w out cols per position
OOFF = np.cumsum([0] + PSTRIP).tolist()
N_UNITS = NPH * BLK

N_WARM = int(os.environ.get("K_WARM", "3"))
EV0A = int(os.environ.get("K_EV0A", "29"))   # of 32 pair evicts (y0) on Act
EV1A = int(os.environ.get("K_EV1A", "0"))   # of 64 single evicts (y1) on Act

LAST_EXEC_TIME_NS = None
_CACHE = {}


def _spread(n_act, total, skip=int(os.environ.get("K_SKIP0", "1"))):
    # spread n_act picks over [skip, total) so the first evictions (pipeline
    # fill, before Act's activation tables are loaded) go to DVE
    if n_act <= 0:
        return set()
    n_act = min(n_act, total - skip)
    return set((skip + np.arange(n_act) * (total - skip) // n_act).tolist())


def _build_program():
    import concourse.bass as bass
    import concourse.bacc as bacc
    import concourse.tile as tile
    from concourse import mybir
    from contextlib import ExitStack

    f32 = mybir.dt.float32
    bf16 = mybir.dt.bfloat16
    Alu = mybir.AluOpType
    Act = mybir.ActivationFunctionType

    nc = bacc.Bacc("TRN2", target_bir_lowering=False, debug=False)

    zd = nc.dram_tensor("z_in", [K0, ZHW], bf16, kind="ExternalInput").ap()
    a0d = nc.dram_tensor("a0t_in", [K0, 128], bf16, kind="ExternalInput").ap()
    w1d = nc.dram_tensor("w1_in", [128, 128], bf16, kind="ExternalInput").ap()
    w2d = nc.dram_tensor("w2_in", [128, 368], bf16, kind="ExternalInput").ap()
    bd = nc.dram_tensor("b_in", [128, 2], f32, kind="ExternalInput").ap()
    outd = nc.dram_tensor("out", [BLK, IPC, OOFF[-1]], bf16,
                          kind="ExternalOutput").ap()

    EV1F = int(os.environ.get("K_EV1F", "0"))
    pat = os.environ.get("K_EV0PAT", "0,4,16,28")
    if pat:
        dve_pairs = set(int(x) for x in pat.split(",") if x != "")
        ACT0 = set(range(N_UNITS // 2)) - dve_pairs
    else:
        ACT0 = _spread(EV0A, N_UNITS // 2)
    ACT1 = _spread(EV1A, N_UNITS, skip=5) | set(range(2, 2 + EV1F))

    U1 = int(os.environ.get("K_U1", "1"))
    U2 = int(os.environ.get("K_U2", "11"))
    U3 = int(os.environ.get("K_U3", "14"))
    D_EV0 = int(os.environ.get("K_DEV0", "1"))
    D_MM1 = int(os.environ.get("K_DMM1", "3"))
    D_EV1 = int(os.environ.get("K_DEV1", "5"))
    D_MM2 = int(os.environ.get("K_DMM2", "7"))
    # engines for [QQ, O_er, O_ee, O_rc]: p=pool, d=dve
    UPS_ENG = os.environ.get("K_UPS", "pppd")
    SY_ACT = os.environ.get("K_SY", "act") == "act"
    SIG_MERGE = os.environ.get("K_SIGM", "0") == "1"

    with tile.TileContext(nc) as tc, ExitStack() as ctx:
        consts = ctx.enter_context(tc.tile_pool(name="consts", bufs=1))
        zt = consts.tile([K0, ZHW], bf16)
        a0 = consts.tile([K0, 128], bf16)
        w1 = consts.tile([128, 128], bf16)
        w2s = consts.tile([128, 368], bf16)
        bb = consts.tile([128, 2], f32)
        warm = consts.tile([K0, 512], bf16)
        scr = consts.tile([1, 8], f32)

        # activation-table preloads first: tiny memset, then one dummy
        # activation per function so the table loads absorb into the idle
        # start instead of blocking the first Act evictions
        nc.vector.memset(scr[:], 0.0)
        nc.scalar.activation(scr[:], scr[:], Act.Sigmoid)
        nc.scalar.activation(scr[:], scr[:], Act.Relu)
        nc.scalar.activation(scr[:], scr[:], Act.Identity)
        nc.vector.memset(warm[:], 0.0)

        # z for the first unit-pair first (mm0 starts ASAP), then consts,
        # then the rest of z
        nc.sync.dma_start(zt[:, 0:ZOFF[1]], zd[:, 0:ZOFF[1]])
        nc.sync.dma_start(a0[:], a0d)
        nc.sync.dma_start(bb[:], bd)
        nc.sync.dma_start(w1[:], w1d)
        nc.sync.dma_start(w2s[:], w2d)
        nc.sync.dma_start(zt[:, ZOFF[1]:ZOFF[4]], zd[:, ZOFF[1]:ZOFF[4]])
        nc.sync.dma_start(zt[:, ZOFF[4]:], zd[:, ZOFF[4]:])

        b1ap = bb[:, 0:1]
        b2ap = bb[:, 1:2]
        w2g = [w2s[:, 240 - 16 * g:368 - 16 * g] for g in range(BLK)]

        p0p = ctx.enter_context(tc.tile_pool(name="p0p", bufs=2, space="PSUM"))
        p1p = ctx.enter_context(tc.tile_pool(name="p1p", bufs=2, space="PSUM"))
        py2p = ctx.enter_context(tc.tile_pool(name="py2p", bufs=2, space="PSUM"))
        y0p = ctx.enter_context(tc.tile_pool(name="y0p", bufs=3))
        y1p = ctx.enter_context(tc.tile_pool(name="y1p", bufs=2))
        syp = ctx.enter_context(tc.tile_pool(name="syp", bufs=3))
        upp = ctx.enter_context(tc.tile_pool(name="upp", bufs=3))
        fop = ctx.enter_context(tc.tile_pool(name="fop", bufs=3))

        # PE warm-up while the first z chunk's DMA is in flight
        for wi in range(N_WARM):
            pw = p0p.tile([128, 1024], f32, tag="p0", name="pw")
            nc.tensor.matmul(pw[:, 0:512], warm[:, 0:128], warm[:],
                             start=True, stop=True)

        state = {}

        def evict(on_act, dst, src, bias_ap):
            if on_act:
                if bias_ap is None:
                    nc.scalar.activation(dst, src, Act.Relu)
                else:
                    nc.scalar.activation(dst, src, Act.Relu, bias=bias_ap)
            else:
                if bias_ap is None:
                    nc.vector.tensor_scalar(dst, src, 0.0, None, Alu.max)
                else:
                    nc.vector.tensor_scalar(dst, src, bias_ap, 0.0,
                                            Alu.add, Alu.max)

        def ucols(p, g):
            # unit 0 of a phase carries the extra duplicate row (18 rows)
            return (RPB + (1 if g == 0 else 0)) * PWE[p]

        def uoff(p, g):
            return ZOFF[p] + (RPB * g + (1 if g else 0)) * PWE[p]

        def s_mm0(u):
            p, g = divmod(u, BLK)
            q = u // 2
            ua, ub = ucols(p, g), ucols(p, g + 1)
            p0t = p0p.tile([128, 1024], f32, tag="p0", name="p0t")
            nc.tensor.matmul(p0t[:, 0:ua], a0[:],
                             zt[:, uoff(p, g):uoff(p, g) + ua],
                             start=True, stop=True)
            nc.tensor.matmul(p0t[:, 512:512 + ub], a0[:],
                             zt[:, uoff(p, g + 1):uoff(p, g + 1) + ub],
                             start=True, stop=True)
            state[("p0", q)] = p0t

        def s_ev0(u):
            p, g = divmod(u, BLK)
            ua, ub = ucols(p, g), ucols(p, g + 1)
            q = u // 2
            p0t = state.pop(("p0", q))
            y0t = y0p.tile([128, 1024], bf16, tag="y0", name="y0t")
            if ua == ub:
                src2 = p0t[:].rearrange("p (b c) -> p b c", b=2)[:, :, 0:ua]
                dst2 = y0t[:].rearrange("p (b c) -> p b c", b=2)[:, :, 0:ua]
                evict(q in ACT0, dst2, src2, None)
            else:
                # unequal halves (18-row unit 0): one 1D evict across both
                # banks, the inter-bank gap is junk
                evict(q in ACT0, y0t[:, 0:512 + ub], p0t[:, 0:512 + ub], None)
            state[("y0", q)] = y0t

        def s_mm1(u):
            p, g = divmod(u, BLK)
            uc = ucols(p, g)
            q, half = divmod(u, 2)
            y0t = state[("y0", q)]
            if half == 1:
                del state[("y0", q)]
            p1t = p1p.tile([128, 512], f32, tag="p1", name="p1t")
            nc.tensor.matmul(p1t[:, 0:uc], w1[:],
                             y0t[:, half * 512:half * 512 + uc],
                             start=True, stop=True)
            state[("p1", u)] = p1t

        def s_ev1(u):
            p, g = divmod(u, BLK)
            pw = PWE[p]
            uc = ucols(p, g)
            p1t = state.pop(("p1", u))
            if g == 0:
                # per-phase y1 tile: slot 0 = the duplicated image row 0
                state[("y1ph", p)] = y1p.tile([128, (H + 1) * 28], bf16,
                                              tag="y1", name="y1t")
            y1t = state[("y1ph", p)]
            off = (RPB * g + (1 if g else 0)) * pw
            evict(u in ACT1, y1t[:, off:off + uc], p1t[:, 0:uc], b1ap)

        def s_mm2(u):
            p, g = divmod(u, BLK)
            pw = PWE[p]
            y1t = state[("y1ph", p)]
            if g == BLK - 1:
                del state[("y1ph", p)]
            if g == 0:
                state[("py2", p)] = py2p.tile([128, 512], f32, tag="py2",
                                              name="py2t")
            py2t = state[("py2", p)]
            # 18-row window: rows 17g-1 .. 17g+16 (slot 17g .. 17g+17)
            nc.tensor.matmul(py2t[:, 0:RPU * pw], w2g[g],
                             y1t[:, RPB * g * pw:(RPB * g + RPU) * pw],
                             start=(g == 0), stop=(g == BLK - 1),
                             skip_group_check=True)

        def s_ups1(p):
            pw = PPW[p]
            ov = POV[p]
            we = PWE[p]
            uc = RPU * we
            sp = PORD[p]                  # spatial phase index
            py2t = state.pop(("py2", p))
            # sy[18, we] = y2 logits (packed copy of py2)
            syt = syp.tile([128, 504], bf16, tag="sy", name="syt")
            sy3 = syt[:, 0:uc].rearrange("q (r c) -> q r c", r=RPU)
            if SY_ACT:
                nc.scalar.activation(syt[:, 0:uc], py2t[:, 0:uc], Act.Identity)
            else:
                nc.vector.tensor_copy(syt[:, 0:uc], py2t[:, 0:uc])
            state[("sy", p)] = syt
            if p >= 2:
                del state[("sy", p - 2)]
            # plane offsets in up/fo: [O_rc 17w][QQ 18w][O_er 17w][O_ee 17w]
            orc, oqq, oer, oee = (0, RPB * pw, (RPB + RPU) * pw,
                                  (2 * RPB + RPU) * pw)
            upt = upp.tile([128, max(PSTRIP)], bf16, tag="up", name="upt")
            state[("up", p)] = upt
            up_qq = upt[:, oqq:oer].rearrange("q (r c) -> q r c", r=RPU)
            lastp = p == NPH - 1
            # QQ[18,pw] = y2[., J-1] + y2[., J]; rows 1..17 are O_re.
            eng = nc.gpsimd if UPS_ENG[0] == "p" and not lastp else nc.vector
            if ov:
                # own leading overlap column: one full-width op
                eng.tensor_tensor(up_qq[:], sy3[:, :, 0:pw],
                                  sy3[:, :, 1:we], Alu.add)
            else:
                eng.tensor_tensor(up_qq[:, :, 1:pw], sy3[:, :, 0:pw - 1],
                                  sy3[:, :, 1:pw], Alu.add)
                if sp == 0:
                    # left image edge: y2[-1] := y2[0]
                    nc.gpsimd.tensor_tensor(up_qq[:, :, 0:1], sy3[:, :, 0:1],
                                            sy3[:, :, 0:1], Alu.add)
                else:
                    pwe = PWE[p - 1]
                    psy3 = state[("sy", p - 1)][:, 0:RPU * pwe].rearrange(
                        "q (r c) -> q r c", r=RPU)
                    nc.gpsimd.tensor_tensor(up_qq[:, :, 0:1],
                                            psy3[:, :, pwe - 1:pwe],
                                            sy3[:, :, 0:1], Alu.add)
            # O_er[17,pw] = y2[k] + y2[k+1]
            eng = nc.gpsimd if UPS_ENG[1] == "p" and not lastp else nc.vector
            if ov:
                eng.tensor_tensor(
                    upt[:, oer:oee].rearrange("q (r c) -> q r c", r=RPB),
                    sy3[:, 0:RPB, 1:we], sy3[:, 1:RPU, 1:we], Alu.add)
            else:
                eng.tensor_tensor(upt[:, oer:oee], syt[:, 0:RPB * pw],
                                  syt[:, pw:uc], Alu.add)
            # O_rc[17,pw] = 2 * y2[1:18]
            eng = nc.gpsimd if UPS_ENG[3] == "p" else nc.vector
            if ov:
                eng.tensor_scalar(
                    upt[:, orc:oqq].rearrange("q (r c) -> q r c", r=RPB),
                    sy3[:, 1:RPU, 1:we], 2.0, None, Alu.mult)
            else:
                eng.tensor_scalar(upt[:, orc:oqq], syt[:, pw:uc], 2.0, None,
                                  Alu.mult)

        def s_ups2(p):
            pw = PPW[p]
            oqq, oer, oee, oend = (RPB * pw, (RPB + RPU) * pw,
                                   (2 * RPB + RPU) * pw, (3 * RPB + RPU) * pw)
            upt = state[("up", p)]
            # O_ee[17,pw] = QQ[k] + QQ[k+1] (4x the logit)
            eng = (nc.gpsimd if UPS_ENG[2] == "p" and p != NPH - 1
                   else nc.vector)
            eng.tensor_tensor(upt[:, oee:oend], upt[:, oqq:oqq + RPB * pw],
                              upt[:, oqq + pw:oer], Alu.add)
            # sigmoids over [O_rc] and [QQ|O_er] (all hold 2x the logit);
            # split so Act's in-order queue never blocks evictions for long
            fot = fop.tile([128, max(PSTRIP)], bf16, tag="fo", name="fot")
            if SIG_MERGE:
                nc.scalar.activation(fot[:, 0:oee], upt[:, 0:oee],
                                     Act.Sigmoid, bias=b2ap, scale=0.5)
            else:
                nc.scalar.activation(fot[:, 0:oqq], upt[:, 0:oqq],
                                     Act.Sigmoid, bias=b2ap, scale=0.5)
            state[("fo", p)] = fot

        def s_ups2b(p):
            pw = PPW[p]
            oqq, oee = RPB * pw, (2 * RPB + RPU) * pw
            upt = state[("up", p)]
            fot = state[("fo", p)]
            if not SIG_MERGE:
                nc.scalar.activation(fot[:, oqq:oee], upt[:, oqq:oee],
                                     Act.Sigmoid, bias=b2ap, scale=0.5)
            dst = outd[:, :, OOFF[p]:OOFF[p] + oee]
            nc.sync.dma_start(dst.rearrange("g i v -> (g i) v"),
                              fot[:, 0:oee])

        def s_ups3(p):
            pw = PPW[p]
            oee, oend = (2 * RPB + RPU) * pw, (3 * RPB + RPU) * pw
            upt = state.pop(("up", p))
            fot = state.pop(("fo", p))
            # O_ee holds 4x the logit
            nc.scalar.activation(fot[:, oee:oend], upt[:, oee:oend],
                                 Act.Sigmoid, bias=b2ap, scale=0.25)
            dst = outd[:, :, OOFF[p] + oee:OOFF[p + 1]]
            nc.sync.dma_start(dst.rearrange("g i v -> (g i) v"),
                              fot[:, oee:oend])

        ups_q = []  # [phase, mm2-done tick, next stage]
        TOTAL = N_UNITS + D_MM2 + U3 + 2
        for i in range(TOTAL):
            if i < N_UNITS and i % 2 == 0:
                s_mm0(i)
            j = i - D_EV0
            if 0 <= j < N_UNITS and j % 2 == 0:
                s_ev0(j)
            j = i - D_MM1
            if 0 <= j < N_UNITS:
                s_mm1(j)
            j = i - D_EV1
            if 0 <= j < N_UNITS:
                s_ev1(j)
            j = i - D_MM2
            if 0 <= j < N_UNITS:
                s_mm2(j)
                if j % BLK == BLK - 1:
                    ups_q.append([j // BLK, i, 1])
            last = i >= TOTAL - 1
            # once every matmul is emitted there is no PE pipeline left to
            # protect from Act head-of-line blocking: flush the remaining
            # upsample stages with tight spacing to shorten the drain tail
            drain = i >= N_UNITS + D_MM2 + int(os.environ.get("K_DREL", "99"))
            UTAIL = int(os.environ.get("K_UTAIL", "99"))
            for item in list(ups_q):
                p, t, st = item
                done = i - t
                if drain or p >= NPH - 2:
                    u2, u2b, u3 = ((2, 3, 4) if drain
                                   else (min(U2, UTAIL), min(U2 + 2, UTAIL + 2),
                                         min(U3, UTAIL + 4)))
                else:
                    u2, u2b, u3 = U2, U2 + 2, U3
                if st == 1 and (done >= U1 or last):
                    s_ups1(p)
                    item[2] = st = 2
                if st == 2 and (done >= u2 or last):
                    s_ups2(p)
                    item[2] = st = 3
                if st == 3 and (done >= u2b or last):
                    s_ups2b(p)
                    item[2] = st = 4
                if st == 4 and (done >= u3 or last):
                    s_ups3(p)
                    ups_q.remove(item)

    nc.compile()
    return nc


def _host_prep(mask_feats, mask_head_params, locations, im_inds, fpn_levels,
               sizes_of_interest):
    import ml_dtypes
    bf16 = ml_dtypes.bfloat16

    mask_feats = np.asarray(mask_feats, dtype=np.float32)
    params = np.asarray(mask_head_params, dtype=np.float32)
    locations = np.asarray(locations, dtype=np.float32)
    im_inds = np.asarray(im_inds).astype(np.int64)
    soi_tab = np.asarray(sizes_of_interest, dtype=np.float32)
    fpn_levels = np.asarray(fpn_levels).astype(np.int64)

    w0 = params[:, 0:80].reshape(N_INST, CH, CIN + 2)
    w1 = params[:, 80:144].reshape(N_INST, CH, CH)
    w2 = params[:, 144:152].reshape(N_INST, 1, CH)
    b0 = params[:, 152:160]
    b1 = params[:, 160:168]
    b2 = params[:, 168:169]

    soi = soi_tab[fpn_levels]
    alpha = -w0[:, :, 0] / soi[:, None]
    beta = -w0[:, :, 1] / soi[:, None]
    c0 = b0 + (w0[:, :, 0] * locations[:, 0:1]
               + w0[:, :, 1] * locations[:, 1:2]) / soi[:, None]
    wfeat = w0[:, :, 2:]

    stride = 8
    xs = np.arange(W, dtype=np.float32) * stride + stride // 2
    ys = np.arange(H, dtype=np.float32) * stride + stride // 2
    z3 = np.empty((K0, H, W), np.float32)
    z3[0] = xs[None, :]
    z3[1] = ys[:, None]
    z3[2] = 1.0
    z3[3:] = mask_feats.reshape(N_IMG * CIN, H, W)
    # strips in PROCESSING order; each strip leads with a duplicate of
    # image row 0 (block 0's edge-pad halo row); the first-processed strip
    # also carries its left-overlap column
    zb = z3[:, np.concatenate([[0], np.arange(H)]), :]        # (K0, 137, W)
    coff = np.cumsum([0] + PHW).tolist()
    strips = []
    for k in range(NPH):
        p = PORD[k]
        s = zb[:, :, coff[p] - POV[k]:coff[p + 1]]            # (K0, 137, we)
        strips.append(s.reshape(K0, (H + 1) * PWE[k]))
    z = np.ascontiguousarray(np.concatenate(strips, axis=1)).astype(bf16)

    in_maps = []
    for c in range(N_CORES):
        a0 = np.zeros((K0, 128), np.float32)
        w1p = np.zeros((128, 128), np.float32)
        w2p = np.zeros((128, 368), np.float32)
        bbv = np.zeros((128, 2), np.float32)
        for i in range(IPC):
            gi = IPC * c + i
            for o in range(CH):
                m = CH * i + o
                a0[0, m] = alpha[gi, o]
                a0[1, m] = beta[gi, o]
                a0[2, m] = c0[gi, o]
                base = 3 + CIN * int(im_inds[gi])
                a0[base:base + CIN, m] = wfeat[gi, o, :]
                w1p[CH * i:CH * i + CH, m] = w1[gi, o, :]
                bbv[m, 0] = b1[gi, o]
            w2p[CH * i:CH * i + CH, 240 + i] = w2[gi, 0, :]
        for q in range(128):
            bbv[q, 1] = b2[IPC * c + (q % IPC), 0]
        in_maps.append({
            "z_in": z,
            "a0t_in": a0.astype(bf16),
            "w1_in": w1p.astype(bf16),
            "w2_in": w2p.astype(bf16),
            "b_in": bbv,
        })
    return in_maps


def kernel(mask_feats, mask_head_params, locations, im_inds, fpn_levels,
           sizes_of_interest, mask_feat_stride):
    global LAST_EXEC_TIME_NS
    assert int(mask_feat_stride) == 8, "kernel hardcodes mask_feat_stride=8"

    os.environ["BASS_NEVER_TRACE"] = "1"
    from concourse.bass_utils import run_bass_kernel_spmd

    in_maps = _host_prep(mask_feats, mask_head_params, locations, im_inds,
                         fpn_levels, sizes_of_interest)

    if "nc" not in _CACHE:
        _CACHE["nc"] = _build_program()
    nc = _CACHE["nc"]

    res = run_bass_kernel_spmd(nc, in_maps, list(range(N_CORES)), trace=False)
    LAST_EXEC_TIME_NS = res.exec_time_ns

    coff = np.cumsum([0] + PHW).tolist()
    out = np.empty((N_INST, 1, OH, OW), np.float32)
    for c in range(N_CORES):
        dev = np.asarray(res.results[c]["out"]).astype(np.float32)
        # dev: [blk g, inst i, strip cols]
        o6 = np.empty((IPC, BLK, RPB, 2, OW), np.float32)
        for k in range(NPH):
            p = PORD[k]
            pw = PHW[p]
            orc, oqq, oer, oee = (np.array([0, RPB, RPB + RPU, 2 * RPB + RPU])
                                  * pw + OOFF[k])
            def plane(off, r):
                pl = dev[:, :, off:off + r * pw].reshape(BLK, IPC, r, pw)
                return pl.transpose(1, 0, 2, 3)
            c0_, c1_ = 2 * coff[p], 2 * coff[p + 1]
            o6[:, :, :, 1, c0_ + 1:c1_:2] = plane(orc, RPB)
            o6[:, :, :, 1, c0_:c1_:2] = plane(oqq, RPU)[:, :, 1:, :]
            o6[:, :, :, 0, c0_ + 1:c1_:2] = plane(oer, RPB)
            o6[:, :, :, 0, c0_:c1_:2] = plane(oee, RPB)
        out[IPC * c:IPC * (c + 1), 0] = o6.transpose(0, 1, 2, 3, 4).reshape(
            IPC, OH, OW)
    return out


# revision 39
# speedup vs baseline: 1.1911x; 1.0008x over previous
"""
Trainium2 Bass kernel for CondConv mask head (CondInst-style dynamic mask head).

Computation (fixed problem size):
  mask_feats (2, 8, 136, 200), 128 instances with per-instance 169 params
  -> per-instance 3-layer 1x1 convs over [rel_coords(2); feats(8)] -> (128,1,136,200)
  -> aligned_bilinear x2 upsample -> sigmoid -> (128, 1, 272, 400)

Strategy (8 NeuronCores, 16 instances per core), v2:
  * All matmul operands are bf16 (1 PE cycle/col).  Host folds rel-coords
    into a shared 19-row spatial matrix Z = [x; y; 1; feats_im0; feats_im1]
    and per-core lhsTs (a0 with the c0 constant on the ones-row,
    block-diagonal w1, and a zero-padded w2 strip whose eight 128-wide
    windows place the w2 block at lhsT columns 16g).
  * The image is processed in 8 column-phases of tunable widths (default
    20,28x6,12 -- narrow last phase to shorten the drain tail); each phase
    is 8 row-block units.  Row blocks OVERLAP by one row (18 rows, width*18
    cols <= 504): block g covers image rows 17g-1 .. 17g+16 (block 0
    edge-pads by duplicating row 0 in Z), so the x2-upsample top halo row
    is computed locally and no cross-partition halo DMA is needed.
  * mm0 writes unit PAIRS into a 2-bank PSUM tile (outs at col 0 and 512 so
    each matmul stays within a bank); the pair is evicted with one relu
    instruction (2D access pattern).  mm1/evict are per-unit; mm2
    accumulates phase logits into py2[128 = 8 blk x 16 inst].  PSUM-reading
    evictions run on ScalarE/VectorE only (GPSIMD has no PSUM port); the
    Act/DVE split is schedule-tuned.  PSUM: p0 2x2 banks + p1 2 + py2 2 = 8.
  * Upsample = 4 polyphase planes, packed bf16 in SBUF (VectorE 2x/4x
    modes, GpSimd helps off the critical chain):
      sy   = py2 copy (packed [18 x W'])
      O_rc = 2*sy[1:18]                 (odd row, odd col)
      QQ   = sy[.,j-1]+sy[.,j] (18 rows; rows 1..17 are the O_re plane;
             col 0 reads the previous phase's last col, tiny separate op)
      O_er = sy[k]+sy[k+1]              (even row, odd col)
      O_ee = QQ[k]+QQ[k+1]              (even row, even col, 4x logit)
  * Sigmoids: one Act instruction over [O_rc|QQ|O_er] (all 2x the logit:
    scale 0.5, bias b2) and one over O_ee (scale 0.25), emitted several
    units later so Act's in-order queue never head-blocks on the planes ->
    contiguous bf16 fo tile -> one 128-descriptor DMA per phase into
    [blk, inst, phase-strip] DRAM.  Host interleaves the planes, f32-casts.
"""

import os
import numpy as np

CH = 8
CIN = 8
N_IMG, H, W = 2, 136, 200
HW = H * W
N_INST = 128
N_CORES = 8
IPC = 16                         # instances per core
FACTOR = 2
OH, OW = H * FACTOR, W * FACTOR  # 272, 400
BLK = 8                          # row-blocks (= units) per phase
RPB = H // BLK                   # 17 output rows per block
RPU = RPB + 1                    # 18 stored rows per unit (one overlap row)
K0 = 3 + N_IMG * CIN             # 19 contraction rows for layer 0

PHW = [int(x) for x in os.environ.get(
    "K_PHW", "6,26,28,28,28,28,28,28").split(",")]
NPH = len(PHW)
assert sum(PHW) == W and all(w <= 28 for w in PHW)
# processing order: rotate so a narrow phase is processed LAST (short drain
# tail).  The first-processed phase carries one extra leading z column (its
# left overlap); every other phase reads its left neighbour's sy, which the
# rotation guarantees was processed just before (spatial phase 0 edge-pads).
ROT = int(os.environ.get("K_ROT", "0")) % NPH
PORD = [(ROT + k) % NPH for k in range(NPH)]
PPW = [PHW[p] for p in PORD]              # width per processing position
POV = [1 if (k == 0 and PORD[0] != 0) else 0 for k in range(NPH)]
PWE = [w + o for w, o in zip(PPW, POV)]   # effective (stored) width
assert all(w <= 28 for w in PWE)
# unit 0 of each phase has 18 rows (leading duplicate of image row 0, the
# edge-pad halo for block 0's 18-row mm2 window)
ZOFF = np.cumsum([0] + [(H + 1) * w for w in PWE]).tolist()
ZHW = ZOFF[-1]
PSTRIP = [(3 * RPB + RPU) * w for w in PPW]   # 69*w out cols per position
OOFF = np.cumsum([0] + PSTRIP).tolist()
N_UNITS = NPH * BLK

N_WARM = int(os.environ.get("K_WARM", "3"))
EV0A = int(os.environ.get("K_EV0A", "29"))   # of 32 pair evicts (y0) on Act
EV1A = int(os.environ.get("K_EV1A", "0"))   # of 64 single evicts (y1) on Act

LAST_EXEC_TIME_NS = None
_CACHE = {}


def _spread(n_act, total, skip=int(os.environ.get("K_SKIP0", "1"))):
    # spread n_act picks over [skip, total) so the first evictions (pipeline
    # fill, before Act's activation tables are loaded) go to DVE
    if n_act <= 0:
        return set()
    n_act = min(n_act, total - skip)
    return set((skip + np.arange(n_act) * (total - skip) // n_act).tolist())


def _build_program():
    import concourse.bass as bass
    import concourse.bacc as bacc
    import concourse.tile as tile
    from concourse import mybir
    from contextlib import ExitStack

    f32 = mybir.dt.float32
    bf16 = mybir.dt.bfloat16
    Alu = mybir.AluOpType
    Act = mybir.ActivationFunctionType

    nc = bacc.Bacc("TRN2", target_bir_lowering=False, debug=False)

    zd = nc.dram_tensor("z_in", [K0, ZHW], bf16, kind="ExternalInput").ap()
    a0d = nc.dram_tensor("a0t_in", [K0, 128], bf16, kind="ExternalInput").ap()
    w1d = nc.dram_tensor("w1_in", [128, 128], bf16, kind="ExternalInput").ap()
    w2d = nc.dram_tensor("w2_in", [128, 368], bf16, kind="ExternalInput").ap()
    bd = nc.dram_tensor("b_in", [128, 2], f32, kind="ExternalInput").ap()
    outd = nc.dram_tensor("out", [BLK, IPC, OOFF[-1]], bf16,
                          kind="ExternalOutput").ap()

    EV1F = int(os.environ.get("K_EV1F", "0"))
    pat = os.environ.get("K_EV0PAT", "0,4,16,28")
    if pat:
        dve_pairs = set(int(x) for x in pat.split(",") if x != "")
        ACT0 = set(range(N_UNITS // 2)) - dve_pairs
    else:
        ACT0 = _spread(EV0A, N_UNITS // 2)
    ACT1 = _spread(EV1A, N_UNITS, skip=5) | set(range(2, 2 + EV1F))

    U1 = int(os.environ.get("K_U1", "1"))
    U2 = int(os.environ.get("K_U2", "11"))
    U3 = int(os.environ.get("K_U3", "14"))
    D_EV0 = int(os.environ.get("K_DEV0", "1"))
    D_MM1 = int(os.environ.get("K_DMM1", "3"))
    D_EV1 = int(os.environ.get("K_DEV1", "5"))
    D_MM2 = int(os.environ.get("K_DMM2", "8"))
    # engines for [QQ, O_er, O_ee, O_rc]: p=pool, d=dve
    UPS_ENG = os.environ.get("K_UPS", "pppd")
    SY_ACT = os.environ.get("K_SY", "act") == "act"
    SIG_MERGE = os.environ.get("K_SIGM", "0") == "1"
    SIG_HALF = os.environ.get("K_SIGH", "0") == "1"
    SY_DVE_P = set(int(x) for x in os.environ.get("K_SYP", "").split(",")
                   if x != "")
    LASTQ = os.environ.get("K_LASTQ", "0") == "1"

    with tile.TileContext(nc) as tc, ExitStack() as ctx:
        consts = ctx.enter_context(tc.tile_pool(name="consts", bufs=1))
        zt = consts.tile([K0, ZHW], bf16)
        a0 = consts.tile([K0, 128], bf16)
        w1 = consts.tile([128, 128], bf16)
        w2s = consts.tile([128, 368], bf16)
        bb = consts.tile([128, 2], f32)
        warm = consts.tile([K0, 512], bf16)
        scr = consts.tile([1, 8], f32)

        # activation-table preloads first: tiny memset, then one dummy
        # activation per function so the table loads absorb into the idle
        # start instead of blocking the first Act evictions
        nc.vector.memset(scr[:], 0.0)
        nc.scalar.activation(scr[:], scr[:], Act.Sigmoid)
        nc.scalar.activation(scr[:], scr[:], Act.Relu)
        nc.scalar.activation(scr[:], scr[:], Act.Identity)
        nc.vector.memset(warm[:], 0.0)

        # z for the first unit-pair first (mm0 starts ASAP), then consts,
        # then the rest of z
        DORD = os.environ.get("K_DORD", "z0,a0,bb,w1,w2,zm,zt")
        dmas = {
            "z0": lambda: nc.sync.dma_start(zt[:, 0:ZOFF[1]], zd[:, 0:ZOFF[1]]),
            "z1": lambda: nc.sync.dma_start(zt[:, ZOFF[1]:ZOFF[2]],
                                            zd[:, ZOFF[1]:ZOFF[2]]),
            "a0": lambda: nc.sync.dma_start(a0[:], a0d),
            "bb": lambda: nc.sync.dma_start(bb[:], bd),
            "w1": lambda: nc.sync.dma_start(w1[:], w1d),
            "w2": lambda: nc.sync.dma_start(w2s[:], w2d),
            "zm": lambda: nc.sync.dma_start(zt[:, ZOFF[1]:ZOFF[4]],
                                            zd[:, ZOFF[1]:ZOFF[4]]),
            "z2": lambda: nc.sync.dma_start(zt[:, ZOFF[2]:ZOFF[4]],
                                            zd[:, ZOFF[2]:ZOFF[4]]),
            "zt": lambda: nc.sync.dma_start(zt[:, ZOFF[4]:], zd[:, ZOFF[4]:]),
        }
        for kk in DORD.split(","):
            dmas[kk]()

        b1ap = bb[:, 0:1]
        b2ap = bb[:, 1:2]
        w2g = [w2s[:, 240 - 16 * g:368 - 16 * g] for g in range(BLK)]

        p0p = ctx.enter_context(tc.tile_pool(name="p0p", bufs=2, space="PSUM"))
        p1p = ctx.enter_context(tc.tile_pool(name="p1p", bufs=2, space="PSUM"))
        py2p = ctx.enter_context(tc.tile_pool(name="py2p", bufs=2, space="PSUM"))
        y0p = ctx.enter_context(tc.tile_pool(name="y0p", bufs=int(os.environ.get("K_Y0B", "3"))))
        y1p = ctx.enter_context(tc.tile_pool(name="y1p", bufs=2))
        syp = ctx.enter_context(tc.tile_pool(name="syp", bufs=3))
        upp = ctx.enter_context(tc.tile_pool(name="upp", bufs=3))
        fop = ctx.enter_context(tc.tile_pool(name="fop", bufs=3))

        # PE warm-up while the first z chunk's DMA is in flight
        for wi in range(N_WARM):
            pw = p0p.tile([128, 1024], f32, tag="p0", name="pw")
            nc.tensor.matmul(pw[:, 0:512], warm[:, 0:128], warm[:],
                             start=True, stop=True)

        state = {}

        def evict(on_act, dst, src, bias_ap):
            if on_act:
                if bias_ap is None:
                    nc.scalar.activation(dst, src, Act.Relu)
                else:
                    nc.scalar.activation(dst, src, Act.Relu, bias=bias_ap)
            else:
                if bias_ap is None:
                    nc.vector.tensor_scalar(dst, src, 0.0, None, Alu.max)
                else:
                    nc.vector.tensor_scalar(dst, src, bias_ap, 0.0,
                                            Alu.add, Alu.max)

        def ucols(p, g):
            # unit 0 of a phase carries the extra duplicate row (18 rows)
            return (RPB + (1 if g == 0 else 0)) * PWE[p]

        def uoff(p, g):
            return ZOFF[p] + (RPB * g + (1 if g else 0)) * PWE[p]

        def s_mm0(u):
            p, g = divmod(u, BLK)
            q = u // 2
            ua, ub = ucols(p, g), ucols(p, g + 1)
            p0t = p0p.tile([128, 1024], f32, tag="p0", name="p0t")
            nc.tensor.matmul(p0t[:, 0:ua], a0[:],
                             zt[:, uoff(p, g):uoff(p, g) + ua],
                             start=True, stop=True)
            nc.tensor.matmul(p0t[:, 512:512 + ub], a0[:],
                             zt[:, uoff(p, g + 1):uoff(p, g + 1) + ub],
                             start=True, stop=True)
            state[("p0", q)] = p0t

        def s_ev0(u):
            p, g = divmod(u, BLK)
            ua, ub = ucols(p, g), ucols(p, g + 1)
            q = u // 2
            p0t = state.pop(("p0", q))
            y0t = y0p.tile([128, 1024], bf16, tag="y0", name="y0t")
            if ua == ub:
                src2 = p0t[:].rearrange("p (b c) -> p b c", b=2)[:, :, 0:ua]
                dst2 = y0t[:].rearrange("p (b c) -> p b c", b=2)[:, :, 0:ua]
                evict(q in ACT0, dst2, src2, None)
            elif 512 - ub <= int(os.environ.get("K_J0", "150")):
                # unequal halves (18-row unit 0): one 1D evict across both
                # banks, the small inter-bank gap is junk
                evict(q in ACT0, y0t[:, 0:512 + ub], p0t[:, 0:512 + ub], None)
            else:
                # narrow phase: the gap would dwarf the data; evict each
                # bank separately
                evict(q in ACT0, y0t[:, 0:ua], p0t[:, 0:ua], None)
                evict(q in ACT0, y0t[:, 512:512 + ub], p0t[:, 512:512 + ub],
                      None)
            state[("y0", q)] = y0t

        def s_mm1(u):
            p, g = divmod(u, BLK)
            uc = ucols(p, g)
            q, half = divmod(u, 2)
            y0t = state[("y0", q)]
            if half == 1:
                del state[("y0", q)]
            p1t = p1p.tile([128, 512], f32, tag="p1", name="p1t")
            nc.tensor.matmul(p1t[:, 0:uc], w1[:],
                             y0t[:, half * 512:half * 512 + uc],
                             start=True, stop=True)
            state[("p1", u)] = p1t

        def s_ev1(u):
            p, g = divmod(u, BLK)
            pw = PWE[p]
            uc = ucols(p, g)
            p1t = state.pop(("p1", u))
            if g == 0:
                # per-phase y1 tile: slot 0 = the duplicated image row 0
                state[("y1ph", p)] = y1p.tile([128, (H + 1) * 28], bf16,
                                              tag="y1", name="y1t")
            y1t = state[("y1ph", p)]
            off = (RPB * g + (1 if g else 0)) * pw
            evict(u in ACT1, y1t[:, off:off + uc], p1t[:, 0:uc], b1ap)

        def s_mm2(u):
            p, g = divmod(u, BLK)
            pw = PWE[p]
            y1t = state[("y1ph", p)]
            if g == BLK - 1:
                del state[("y1ph", p)]
            if g == 0:
                state[("py2", p)] = py2p.tile([128, 512], f32, tag="py2",
                                              name="py2t")
            py2t = state[("py2", p)]
            # 18-row window: rows 17g-1 .. 17g+16 (slot 17g .. 17g+17)
            nc.tensor.matmul(py2t[:, 0:RPU * pw], w2g[g],
                             y1t[:, RPB * g * pw:(RPB * g + RPU) * pw],
                             start=(g == 0), stop=(g == BLK - 1),
                             skip_group_check=True)

        def s_ups1(p):
            pw = PPW[p]
            ov = POV[p]
            we = PWE[p]
            uc = RPU * we
            sp = PORD[p]                  # spatial phase index
            py2t = state.pop(("py2", p))
            # sy[18, we] = y2 logits (packed copy of py2)
            syt = syp.tile([128, 504], bf16, tag="sy", name="syt")
            sy3 = syt[:, 0:uc].rearrange("q (r c) -> q r c", r=RPU)
            if SY_ACT and p not in SY_DVE_P:
                nc.scalar.activation(syt[:, 0:uc], py2t[:, 0:uc], Act.Identity)
            else:
                nc.vector.tensor_copy(syt[:, 0:uc], py2t[:, 0:uc])
            state[("sy", p)] = syt
            if p >= 2:
                del state[("sy", p - 2)]
            # plane offsets in up/fo: [O_rc 17w][QQ 18w][O_er 17w][O_ee 17w]
            orc, oqq, oer, oee = (0, RPB * pw, (RPB + RPU) * pw,
                                  (2 * RPB + RPU) * pw)
            upt = upp.tile([128, max(PSTRIP)], bf16, tag="up", name="upt")
            state[("up", p)] = upt
            up_qq = upt[:, oqq:oer].rearrange("q (r c) -> q r c", r=RPU)
            lastp = p == NPH - 1
            # QQ[18,pw] = y2[., J-1] + y2[., J]; rows 1..17 are O_re.
            eng = nc.gpsimd if UPS_ENG[0] == "p" and not lastp else nc.vector
            if ov:
                # own leading overlap column: one full-width op
                eng.tensor_tensor(up_qq[:], sy3[:, :, 0:pw],
                                  sy3[:, :, 1:we], Alu.add)
            else:
                eng.tensor_tensor(up_qq[:, :, 1:pw], sy3[:, :, 0:pw - 1],
                                  sy3[:, :, 1:pw], Alu.add)
                if sp == 0:
                    # left image edge: y2[-1] := y2[0]
                    nc.gpsimd.tensor_tensor(up_qq[:, :, 0:1], sy3[:, :, 0:1],
                                            sy3[:, :, 0:1], Alu.add)
                else:
                    pwe = PWE[p - 1]
                    psy3 = state[("sy", p - 1)][:, 0:RPU * pwe].rearrange(
                        "q (r c) -> q r c", r=RPU)
                    nc.gpsimd.tensor_tensor(up_qq[:, :, 0:1],
                                            psy3[:, :, pwe - 1:pwe],
                                            sy3[:, :, 0:1], Alu.add)
            # O_er[17,pw] = y2[k] + y2[k+1]
            eng = nc.gpsimd if UPS_ENG[1] == "p" and not lastp else nc.vector
            if ov:
                eng.tensor_tensor(
                    upt[:, oer:oee].rearrange("q (r c) -> q r c", r=RPB),
                    sy3[:, 0:RPB, 1:we], sy3[:, 1:RPU, 1:we], Alu.add)
            else:
                eng.tensor_tensor(upt[:, oer:oee], syt[:, 0:RPB * pw],
                                  syt[:, pw:uc], Alu.add)
            # O_rc[17,pw] = 2 * y2[1:18]
            eng = nc.gpsimd if UPS_ENG[3] == "p" else nc.vector
            if ov:
                eng.tensor_scalar(
                    upt[:, orc:oqq].rearrange("q (r c) -> q r c", r=RPB),
                    sy3[:, 1:RPU, 1:we], 2.0, None, Alu.mult)
            else:
                eng.tensor_scalar(upt[:, orc:oqq], syt[:, pw:uc], 2.0, None,
                                  Alu.mult)

        def s_ups2(p):
            pw = PPW[p]
            oqq, oer, oee, oend = (RPB * pw, (RPB + RPU) * pw,
                                   (2 * RPB + RPU) * pw, (3 * RPB + RPU) * pw)
            upt = state[("up", p)]
            # O_ee[17,pw] = QQ[k] + QQ[k+1] (4x the logit)
            eng = (nc.gpsimd if UPS_ENG[2] == "p" and p != NPH - 1
                   else nc.vector)
            eng.tensor_tensor(upt[:, oee:oend], upt[:, oqq:oqq + RPB * pw],
                              upt[:, oqq + pw:oer], Alu.add)
            # sigmoids over [O_rc] and [QQ|O_er] (all hold 2x the logit);
            # split so Act's in-order queue never blocks evictions for long
            fot = fop.tile([128, max(PSTRIP)], bf16, tag="fo", name="fot")
            # split point: halve the [O_rc|QQ|O_er] range so neither piece
            # head-of-line-blocks Act's queue for long (sigmoid is
            # elementwise, any column split is valid)
            osp = (oee // (2 * pw)) * pw if SIG_HALF else oqq
            if SIG_MERGE:
                nc.scalar.activation(fot[:, 0:oee], upt[:, 0:oee],
                                     Act.Sigmoid, bias=b2ap, scale=0.5)
            else:
                nc.scalar.activation(fot[:, 0:osp], upt[:, 0:osp],
                                     Act.Sigmoid, bias=b2ap, scale=0.5)
            state[("fo", p)] = fot

        def s_ups2b(p):
            pw = PPW[p]
            oqq, oee = RPB * pw, (2 * RPB + RPU) * pw
            osp = (oee // (2 * pw)) * pw if SIG_HALF else oqq
            upt = state[("up", p)]
            fot = state[("fo", p)]
            if not SIG_MERGE:
                nc.scalar.activation(fot[:, osp:oee], upt[:, osp:oee],
                                     Act.Sigmoid, bias=b2ap, scale=0.5)
            dst = outd[:, :, OOFF[p]:OOFF[p] + oee]
            # last phase: use the scalar queue so this dispatch overlaps the
            # final O_ee DMA's on the sync queue (nothing left on Act after)
            q = nc.scalar if (p == NPH - 1 and LASTQ) else nc.sync
            q.dma_start(dst.rearrange("g i v -> (g i) v"), fot[:, 0:oee])

        def s_ups3(p):
            pw = PPW[p]
            oee, oend = (2 * RPB + RPU) * pw, (3 * RPB + RPU) * pw
            upt = state.pop(("up", p))
            fot = state.pop(("fo", p))
            # O_ee holds 4x the logit
            nc.scalar.activation(fot[:, oee:oend], upt[:, oee:oend],
                                 Act.Sigmoid, bias=b2ap, scale=0.25)
            dst = outd[:, :, OOFF[p] + oee:OOFF[p + 1]]
            nc.sync.dma_start(dst.rearrange("g i v -> (g i) v"),
                              fot[:, oee:oend])

        ups_q = []  # [phase, mm2-done tick, next stage]
        TOTAL = N_UNITS + D_MM2 + U3 + 2
        OLDF = os.environ.get("K_OLDF", "0") == "1"
        for i in range(TOTAL):
            if OLDF:
                j = i - D_MM2
                if 0 <= j < N_UNITS:
                    s_mm2(j)
                    if j % BLK == BLK - 1:
                        ups_q.append([j // BLK, i, 1])
                j = i - D_EV1
                if 0 <= j < N_UNITS:
                    s_ev1(j)
                j = i - D_MM1
                if 0 <= j < N_UNITS:
                    s_mm1(j)
                j = i - D_EV0
                if 0 <= j < N_UNITS and j % 2 == 0:
                    s_ev0(j)
                if i < N_UNITS and i % 2 == 0:
                    s_mm0(i)
            else:
                if i < N_UNITS and i % 2 == 0:
                    s_mm0(i)
                j = i - D_EV0
                if 0 <= j < N_UNITS and j % 2 == 0:
                    s_ev0(j)
                j = i - D_MM1
                if 0 <= j < N_UNITS:
                    s_mm1(j)
                j = i - D_EV1
                if 0 <= j < N_UNITS:
                    s_ev1(j)
                j = i - D_MM2
                if 0 <= j < N_UNITS:
                    s_mm2(j)
                    if j % BLK == BLK - 1:
                        ups_q.append([j // BLK, i, 1])
            last = i >= TOTAL - 1
            # once every matmul is emitted there is no PE pipeline left to
            # protect from Act head-of-line blocking: flush the remaining
            # upsample stages with tight spacing to shorten the drain tail
            drain = i >= N_UNITS + D_MM2 + int(os.environ.get("K_DREL", "99"))
            UTAIL = int(os.environ.get("K_UTAIL", "99"))
            for item in list(ups_q):
                p, t, st = item
                done = i - t
                if drain or p >= NPH - 2:
                    u2, u2b, u3 = ((2, 3, 4) if drain
                                   else (min(U2, UTAIL), min(U2 + 2, UTAIL + 2),
                                         min(U3, UTAIL + 4)))
                else:
                    u2, u2b, u3 = U2, U2 + 2, U3
                if st == 1 and (done >= U1 or last):
                    s_ups1(p)
                    item[2] = st = 2
                if st == 2 and (done >= u2 or last):
                    s_ups2(p)
                    item[2] = st = 3
                if st == 3 and (done >= u2b or last):
                    s_ups2b(p)
                    item[2] = st = 4
                if st == 4 and (done >= u3 or last):
                    s_ups3(p)
                    ups_q.remove(item)

    nc.compile()
    return nc


def _host_prep(mask_feats, mask_head_params, locations, im_inds, fpn_levels,
               sizes_of_interest):
    import ml_dtypes
    bf16 = ml_dtypes.bfloat16

    mask_feats = np.asarray(mask_feats, dtype=np.float32)
    params = np.asarray(mask_head_params, dtype=np.float32)
    locations = np.asarray(locations, dtype=np.float32)
    im_inds = np.asarray(im_inds).astype(np.int64)
    soi_tab = np.asarray(sizes_of_interest, dtype=np.float32)
    fpn_levels = np.asarray(fpn_levels).astype(np.int64)

    w0 = params[:, 0:80].reshape(N_INST, CH, CIN + 2)
    w1 = params[:, 80:144].reshape(N_INST, CH, CH)
    w2 = params[:, 144:152].reshape(N_INST, 1, CH)
    b0 = params[:, 152:160]
    b1 = params[:, 160:168]
    b2 = params[:, 168:169]

    soi = soi_tab[fpn_levels]
    alpha = -w0[:, :, 0] / soi[:, None]
    beta = -w0[:, :, 1] / soi[:, None]
    c0 = b0 + (w0[:, :, 0] * locations[:, 0:1]
               + w0[:, :, 1] * locations[:, 1:2]) / soi[:, None]
    wfeat = w0[:, :, 2:]

    stride = 8
    xs = np.arange(W, dtype=np.float32) * stride + stride // 2
    ys = np.arange(H, dtype=np.float32) * stride + stride // 2
    z3 = np.empty((K0, H, W), np.float32)
    z3[0] = xs[None, :]
    z3[1] = ys[:, None]
    z3[2] = 1.0
    z3[3:] = mask_feats.reshape(N_IMG * CIN, H, W)
    # strips in PROCESSING order; each strip leads with a duplicate of
    # image row 0 (block 0's edge-pad halo row); the first-processed strip
    # also carries its left-overlap column
    zb = z3[:, np.concatenate([[0], np.arange(H)]), :]        # (K0, 137, W)
    coff = np.cumsum([0] + PHW).tolist()
    strips = []
    for k in range(NPH):
        p = PORD[k]
        s = zb[:, :, coff[p] - POV[k]:coff[p + 1]]            # (K0, 137, we)
        strips.append(s.reshape(K0, (H + 1) * PWE[k]))
    z = np.ascontiguousarray(np.concatenate(strips, axis=1)).astype(bf16)

    in_maps = []
    for c in range(N_CORES):
        a0 = np.zeros((K0, 128), np.float32)
        w1p = np.zeros((128, 128), np.float32)
        w2p = np.zeros((128, 368), np.float32)
        bbv = np.zeros((128, 2), np.float32)
        for i in range(IPC):
            gi = IPC * c + i
            for o in range(CH):
                m = CH * i + o
                a0[0, m] = alpha[gi, o]
                a0[1, m] = beta[gi, o]
                a0[2, m] = c0[gi, o]
                base = 3 + CIN * int(im_inds[gi])
                a0[base:base + CIN, m] = wfeat[gi, o, :]
                w1p[CH * i:CH * i + CH, m] = w1[gi, o, :]
                bbv[m, 0] = b1[gi, o]
            w2p[CH * i:CH * i + CH, 240 + i] = w2[gi, 0, :]
        for q in range(128):
            bbv[q, 1] = b2[IPC * c + (q % IPC), 0]
        in_maps.append({
            "z_in": z,
            "a0t_in": a0.astype(bf16),
            "w1_in": w1p.astype(bf16),
            "w2_in": w2p.astype(bf16),
            "b_in": bbv,
        })
    return in_maps


def kernel(mask_feats, mask_head_params, locations, im_inds, fpn_levels,
           sizes_of_interest, mask_feat_stride):
    global LAST_EXEC_TIME_NS
    assert int(mask_feat_stride) == 8, "kernel hardcodes mask_feat_stride=8"

    os.environ["BASS_NEVER_TRACE"] = "1"
    from concourse.bass_utils import run_bass_kernel_spmd

    in_maps = _host_prep(mask_feats, mask_head_params, locations, im_inds,
                         fpn_levels, sizes_of_interest)

    if "nc" not in _CACHE:
        _CACHE["nc"] = _build_program()
    nc = _CACHE["nc"]

    res = run_bass_kernel_spmd(nc, in_maps, list(range(N_CORES)), trace=False)
    LAST_EXEC_TIME_NS = res.exec_time_ns

    coff = np.cumsum([0] + PHW).tolist()
    out = np.empty((N_INST, 1, OH, OW), np.float32)
    for c in range(N_CORES):
        dev = np.asarray(res.results[c]["out"]).astype(np.float32)
        # dev: [blk g, inst i, strip cols]
        o6 = np.empty((IPC, BLK, RPB, 2, OW), np.float32)
        for k in range(NPH):
            p = PORD[k]
            pw = PHW[p]
            orc, oqq, oer, oee = (np.array([0, RPB, RPB + RPU, 2 * RPB + RPU])
                                  * pw + OOFF[k])
            def plane(off, r):
                pl = dev[:, :, off:off + r * pw].reshape(BLK, IPC, r, pw)
                return pl.transpose(1, 0, 2, 3)
            c0_, c1_ = 2 * coff[p], 2 * coff[p + 1]
            o6[:, :, :, 1, c0_ + 1:c1_:2] = plane(orc, RPB)
            o6[:, :, :, 1, c0_:c1_:2] = plane(oqq, RPU)[:, :, 1:, :]
            o6[:, :, :, 0, c0_ + 1:c1_:2] = plane(oer, RPB)
            o6[:, :, :, 0, c0_:c1_:2] = plane(oee, RPB)
        out[IPC * c:IPC * (c + 1), 0] = o6.transpose(0, 1, 2, 3, 4).reshape(
            IPC, OH, OW)
    return out


# revision 40
# speedup vs baseline: 1.1932x; 1.0018x over previous
"""
Trainium2 Bass kernel for CondConv mask head (CondInst-style dynamic mask head).

Computation (fixed problem size):
  mask_feats (2, 8, 136, 200), 128 instances with per-instance 169 params
  -> per-instance 3-layer 1x1 convs over [rel_coords(2); feats(8)] -> (128,1,136,200)
  -> aligned_bilinear x2 upsample -> sigmoid -> (128, 1, 272, 400)

Strategy (8 NeuronCores, 16 instances per core), v2:
  * All matmul operands are bf16 (1 PE cycle/col).  Host folds rel-coords
    into a shared 19-row spatial matrix Z = [x; y; 1; feats_im0; feats_im1]
    and per-core lhsTs (a0 with the c0 constant on the ones-row,
    block-diagonal w1, and a zero-padded w2 strip whose eight 128-wide
    windows place the w2 block at lhsT columns 16g).
  * The image is processed in 8 column-phases of tunable widths (default
    20,28x6,12 -- narrow last phase to shorten the drain tail); each phase
    is 8 row-block units.  Row blocks OVERLAP by one row (18 rows, width*18
    cols <= 504): block g covers image rows 17g-1 .. 17g+16 (block 0
    edge-pads by duplicating row 0 in Z), so the x2-upsample top halo row
    is computed locally and no cross-partition halo DMA is needed.
  * mm0 writes unit PAIRS into a 2-bank PSUM tile (outs at col 0 and 512 so
    each matmul stays within a bank); the pair is evicted with one relu
    instruction (2D access pattern).  mm1/evict are per-unit; mm2
    accumulates phase logits into py2[128 = 8 blk x 16 inst].  PSUM-reading
    evictions run on ScalarE/VectorE only (GPSIMD has no PSUM port); the
    Act/DVE split is schedule-tuned.  PSUM: p0 2x2 banks + p1 2 + py2 2 = 8.
  * Upsample = 4 polyphase planes, packed bf16 in SBUF (VectorE 2x/4x
    modes, GpSimd helps off the critical chain):
      sy   = py2 copy (packed [18 x W'])
      O_rc = 2*sy[1:18]                 (odd row, odd col)
      QQ   = sy[.,j-1]+sy[.,j] (18 rows; rows 1..17 are the O_re plane;
             col 0 reads the previous phase's last col, tiny separate op)
      O_er = sy[k]+sy[k+1]              (even row, odd col)
      O_ee = QQ[k]+QQ[k+1]              (even row, even col, 4x logit)
  * Sigmoids: one Act instruction over [O_rc|QQ|O_er] (all 2x the logit:
    scale 0.5, bias b2) and one over O_ee (scale 0.25), emitted several
    units later so Act's in-order queue never head-blocks on the planes ->
    contiguous bf16 fo tile -> one 128-descriptor DMA per phase into
    [blk, inst, phase-strip] DRAM.  Host interleaves the planes, f32-casts.
"""

import os
import numpy as np

CH = 8
CIN = 8
N_IMG, H, W = 2, 136, 200
HW = H * W
N_INST = 128
N_CORES = 8
IPC = 16                         # instances per core
FACTOR = 2
OH, OW = H * FACTOR, W * FACTOR  # 272, 400
BLK = 8                          # row-blocks (= units) per phase
RPB = H // BLK                   # 17 output rows per block
RPU = RPB + 1                    # 18 stored rows per unit (one overlap row)
K0 = 3 + N_IMG * CIN             # 19 contraction rows for layer 0

PHW = [int(x) for x in os.environ.get(
    "K_PHW", "6,26,28,28,28,28,28,28").split(",")]
NPH = len(PHW)
assert sum(PHW) == W and all(w <= 28 for w in PHW)
# processing order: rotate so a narrow phase is processed LAST (short drain
# tail).  The first-processed phase carries one extra leading z column (its
# left overlap); every other phase reads its left neighbour's sy, which the
# rotation guarantees was processed just before (spatial phase 0 edge-pads).
ROT = int(os.environ.get("K_ROT", "0")) % NPH
PORD = [(ROT + k) % NPH for k in range(NPH)]
PPW = [PHW[p] for p in PORD]              # width per processing position
POV = [1 if (k == 0 and PORD[0] != 0) else 0 for k in range(NPH)]
PWE = [w + o for w, o in zip(PPW, POV)]   # effective (stored) width
assert all(w <= 28 for w in PWE)
# unit 0 of each phase has 18 rows (leading duplicate of image row 0, the
# edge-pad halo for block 0's 18-row mm2 window)
ZOFF = np.cumsum([0] + [(H + 1) * w for w in PWE]).tolist()
ZHW = ZOFF[-1]
PSTRIP = [(3 * RPB + RPU) * w for w in PPW]   # 69*w out cols per position
OOFF = np.cumsum([0] + PSTRIP).tolist()
N_UNITS = NPH * BLK

N_WARM = int(os.environ.get("K_WARM", "3"))
EV0A = int(os.environ.get("K_EV0A", "29"))   # of 32 pair evicts (y0) on Act
EV1A = int(os.environ.get("K_EV1A", "0"))   # of 64 single evicts (y1) on Act

LAST_EXEC_TIME_NS = None
_CACHE = {}


def _spread(n_act, total, skip=int(os.environ.get("K_SKIP0", "1"))):
    # spread n_act picks over [skip, total) so the first evictions (pipeline
    # fill, before Act's activation tables are loaded) go to DVE
    if n_act <= 0:
        return set()
    n_act = min(n_act, total - skip)
    return set((skip + np.arange(n_act) * (total - skip) // n_act).tolist())


def _build_program():
    import concourse.bass as bass
    import concourse.bacc as bacc
    import concourse.tile as tile
    from concourse import mybir
    from contextlib import ExitStack

    f32 = mybir.dt.float32
    bf16 = mybir.dt.bfloat16
    Alu = mybir.AluOpType
    Act = mybir.ActivationFunctionType

    nc = bacc.Bacc("TRN2", target_bir_lowering=False, debug=False)

    zd = nc.dram_tensor("z_in", [K0, ZHW], bf16, kind="ExternalInput").ap()
    a0d = nc.dram_tensor("a0t_in", [K0, 128], bf16, kind="ExternalInput").ap()
    w1d = nc.dram_tensor("w1_in", [128, 128], bf16, kind="ExternalInput").ap()
    w2d = nc.dram_tensor("w2_in", [128, 368], bf16, kind="ExternalInput").ap()
    bd = nc.dram_tensor("b_in", [128, 2], f32, kind="ExternalInput").ap()
    outd = nc.dram_tensor("out", [BLK, IPC, OOFF[-1]], bf16,
                          kind="ExternalOutput").ap()

    EV1F = int(os.environ.get("K_EV1F", "0"))
    pat = os.environ.get("K_EV0PAT", "0,4,20,28")
    if pat:
        dve_pairs = set(int(x) for x in pat.split(",") if x != "")
        ACT0 = set(range(N_UNITS // 2)) - dve_pairs
    else:
        ACT0 = _spread(EV0A, N_UNITS // 2)
    ACT1 = _spread(EV1A, N_UNITS, skip=5) | set(range(2, 2 + EV1F))

    U1 = int(os.environ.get("K_U1", "1"))
    U2 = int(os.environ.get("K_U2", "11"))
    U3 = int(os.environ.get("K_U3", "14"))
    D_EV0 = int(os.environ.get("K_DEV0", "1"))
    D_MM1 = int(os.environ.get("K_DMM1", "3"))
    D_EV1 = int(os.environ.get("K_DEV1", "5"))
    D_MM2 = int(os.environ.get("K_DMM2", "8"))
    # engines for [QQ, O_er, O_ee, O_rc]: p=pool, d=dve
    UPS_ENG = os.environ.get("K_UPS", "pppd")
    SY_ACT = os.environ.get("K_SY", "act") == "act"
    SIG_MERGE = os.environ.get("K_SIGM", "0") == "1"
    SIG_HALF = os.environ.get("K_SIGH", "0") == "1"
    SY_DVE_P = set(int(x) for x in os.environ.get("K_SYP", "").split(",")
                   if x != "")
    LASTQ = os.environ.get("K_LASTQ", "0") == "1"

    with tile.TileContext(nc) as tc, ExitStack() as ctx:
        consts = ctx.enter_context(tc.tile_pool(name="consts", bufs=1))
        zt = consts.tile([K0, ZHW], bf16)
        a0 = consts.tile([K0, 128], bf16)
        w1 = consts.tile([128, 128], bf16)
        w2s = consts.tile([128, 368], bf16)
        bb = consts.tile([128, 2], f32)
        warm = consts.tile([K0, 512], bf16)
        scr = consts.tile([1, 8], f32)

        # activation-table preloads first: tiny memset, then one dummy
        # activation per function so the table loads absorb into the idle
        # start instead of blocking the first Act evictions
        nc.vector.memset(scr[:], 0.0)
        nc.scalar.activation(scr[:], scr[:], Act.Sigmoid)
        nc.scalar.activation(scr[:], scr[:], Act.Relu)
        nc.scalar.activation(scr[:], scr[:], Act.Identity)
        nc.vector.memset(warm[:], 0.0)

        # z for the first unit-pair first (mm0 starts ASAP), then consts,
        # then the rest of z
        DORD = os.environ.get("K_DORD", "z0,a0,bb,w1,w2,zm,zt")
        dmas = {
            "z0": lambda: nc.sync.dma_start(zt[:, 0:ZOFF[1]], zd[:, 0:ZOFF[1]]),
            "z1": lambda: nc.sync.dma_start(zt[:, ZOFF[1]:ZOFF[2]],
                                            zd[:, ZOFF[1]:ZOFF[2]]),
            "a0": lambda: nc.sync.dma_start(a0[:], a0d),
            "bb": lambda: nc.sync.dma_start(bb[:], bd),
            "w1": lambda: nc.sync.dma_start(w1[:], w1d),
            "w2": lambda: nc.sync.dma_start(w2s[:], w2d),
            "zm": lambda: nc.sync.dma_start(zt[:, ZOFF[1]:ZOFF[4]],
                                            zd[:, ZOFF[1]:ZOFF[4]]),
            "z2": lambda: nc.sync.dma_start(zt[:, ZOFF[2]:ZOFF[4]],
                                            zd[:, ZOFF[2]:ZOFF[4]]),
            "zt": lambda: nc.sync.dma_start(zt[:, ZOFF[4]:], zd[:, ZOFF[4]:]),
        }
        for kk in DORD.split(","):
            dmas[kk]()

        b1ap = bb[:, 0:1]
        b2ap = bb[:, 1:2]
        w2g = [w2s[:, 240 - 16 * g:368 - 16 * g] for g in range(BLK)]

        p0p = ctx.enter_context(tc.tile_pool(name="p0p", bufs=2, space="PSUM"))
        p1p = ctx.enter_context(tc.tile_pool(name="p1p", bufs=2, space="PSUM"))
        py2p = ctx.enter_context(tc.tile_pool(name="py2p", bufs=2, space="PSUM"))
        y0p = ctx.enter_context(tc.tile_pool(name="y0p", bufs=int(os.environ.get("K_Y0B", "3"))))
        y1p = ctx.enter_context(tc.tile_pool(name="y1p", bufs=2))
        syp = ctx.enter_context(tc.tile_pool(name="syp", bufs=3))
        upp = ctx.enter_context(tc.tile_pool(name="upp", bufs=3))
        fop = ctx.enter_context(tc.tile_pool(name="fop", bufs=3))

        # PE warm-up while the first z chunk's DMA is in flight
        for wi in range(N_WARM):
            pw = p0p.tile([128, 1024], f32, tag="p0", name="pw")
            nc.tensor.matmul(pw[:, 0:512], warm[:, 0:128], warm[:],
                             start=True, stop=True)

        state = {}

        def evict(on_act, dst, src, bias_ap):
            if on_act:
                if bias_ap is None:
                    nc.scalar.activation(dst, src, Act.Relu)
                else:
                    nc.scalar.activation(dst, src, Act.Relu, bias=bias_ap)
            else:
                if bias_ap is None:
                    nc.vector.tensor_scalar(dst, src, 0.0, None, Alu.max)
                else:
                    nc.vector.tensor_scalar(dst, src, bias_ap, 0.0,
                                            Alu.add, Alu.max)

        def ucols(p, g):
            # unit 0 of a phase carries the extra duplicate row (18 rows)
            return (RPB + (1 if g == 0 else 0)) * PWE[p]

        def uoff(p, g):
            return ZOFF[p] + (RPB * g + (1 if g else 0)) * PWE[p]

        def s_mm0(u):
            p, g = divmod(u, BLK)
            q = u // 2
            ua, ub = ucols(p, g), ucols(p, g + 1)
            p0t = p0p.tile([128, 1024], f32, tag="p0", name="p0t")
            nc.tensor.matmul(p0t[:, 0:ua], a0[:],
                             zt[:, uoff(p, g):uoff(p, g) + ua],
                             start=True, stop=True)
            nc.tensor.matmul(p0t[:, 512:512 + ub], a0[:],
                             zt[:, uoff(p, g + 1):uoff(p, g + 1) + ub],
                             start=True, stop=True)
            state[("p0", q)] = p0t

        def s_ev0(u):
            p, g = divmod(u, BLK)
            ua, ub = ucols(p, g), ucols(p, g + 1)
            q = u // 2
            p0t = state.pop(("p0", q))
            y0t = y0p.tile([128, 1024], bf16, tag="y0", name="y0t")
            if ua == ub:
                src2 = p0t[:].rearrange("p (b c) -> p b c", b=2)[:, :, 0:ua]
                dst2 = y0t[:].rearrange("p (b c) -> p b c", b=2)[:, :, 0:ua]
                evict(q in ACT0, dst2, src2, None)
            elif 512 - ub <= int(os.environ.get("K_J0", "150")):
                # unequal halves (18-row unit 0): one 1D evict across both
                # banks, the small inter-bank gap is junk
                evict(q in ACT0, y0t[:, 0:512 + ub], p0t[:, 0:512 + ub], None)
            else:
                # narrow phase: the gap would dwarf the data; evict each
                # bank separately
                evict(q in ACT0, y0t[:, 0:ua], p0t[:, 0:ua], None)
                evict(q in ACT0, y0t[:, 512:512 + ub], p0t[:, 512:512 + ub],
                      None)
            state[("y0", q)] = y0t

        def s_mm1(u):
            p, g = divmod(u, BLK)
            uc = ucols(p, g)
            q, half = divmod(u, 2)
            y0t = state[("y0", q)]
            if half == 1:
                del state[("y0", q)]
            p1t = p1p.tile([128, 512], f32, tag="p1", name="p1t")
            nc.tensor.matmul(p1t[:, 0:uc], w1[:],
                             y0t[:, half * 512:half * 512 + uc],
                             start=True, stop=True)
            state[("p1", u)] = p1t

        def s_ev1(u):
            p, g = divmod(u, BLK)
            pw = PWE[p]
            uc = ucols(p, g)
            p1t = state.pop(("p1", u))
            if g == 0:
                # per-phase y1 tile: slot 0 = the duplicated image row 0
                state[("y1ph", p)] = y1p.tile([128, (H + 1) * 28], bf16,
                                              tag="y1", name="y1t")
            y1t = state[("y1ph", p)]
            off = (RPB * g + (1 if g else 0)) * pw
            evict(u in ACT1, y1t[:, off:off + uc], p1t[:, 0:uc], b1ap)

        def s_mm2(u):
            p, g = divmod(u, BLK)
            pw = PWE[p]
            y1t = state[("y1ph", p)]
            if g == BLK - 1:
                del state[("y1ph", p)]
            if g == 0:
                state[("py2", p)] = py2p.tile([128, 512], f32, tag="py2",
                                              name="py2t")
            py2t = state[("py2", p)]
            # 18-row window: rows 17g-1 .. 17g+16 (slot 17g .. 17g+17)
            nc.tensor.matmul(py2t[:, 0:RPU * pw], w2g[g],
                             y1t[:, RPB * g * pw:(RPB * g + RPU) * pw],
                             start=(g == 0), stop=(g == BLK - 1),
                             skip_group_check=True)

        def s_ups1(p):
            pw = PPW[p]
            ov = POV[p]
            we = PWE[p]
            uc = RPU * we
            sp = PORD[p]                  # spatial phase index
            py2t = state.pop(("py2", p))
            # sy[18, we] = y2 logits (packed copy of py2)
            syt = syp.tile([128, 504], bf16, tag="sy", name="syt")
            sy3 = syt[:, 0:uc].rearrange("q (r c) -> q r c", r=RPU)
            if SY_ACT and p not in SY_DVE_P:
                nc.scalar.activation(syt[:, 0:uc], py2t[:, 0:uc], Act.Identity)
            else:
                nc.vector.tensor_copy(syt[:, 0:uc], py2t[:, 0:uc])
            state[("sy", p)] = syt
            if p >= 2:
                del state[("sy", p - 2)]
            # plane offsets in up/fo: [O_rc 17w][QQ 18w][O_er 17w][O_ee 17w]
            orc, oqq, oer, oee = (0, RPB * pw, (RPB + RPU) * pw,
                                  (2 * RPB + RPU) * pw)
            upt = upp.tile([128, max(PSTRIP)], bf16, tag="up", name="upt")
            state[("up", p)] = upt
            up_qq = upt[:, oqq:oer].rearrange("q (r c) -> q r c", r=RPU)
            lastp = p == NPH - 1
            # QQ[18,pw] = y2[., J-1] + y2[., J]; rows 1..17 are O_re.
            eng = nc.gpsimd if UPS_ENG[0] == "p" and not lastp else nc.vector
            if ov:
                # own leading overlap column: one full-width op
                eng.tensor_tensor(up_qq[:], sy3[:, :, 0:pw],
                                  sy3[:, :, 1:we], Alu.add)
            else:
                eng.tensor_tensor(up_qq[:, :, 1:pw], sy3[:, :, 0:pw - 1],
                                  sy3[:, :, 1:pw], Alu.add)
                if sp == 0:
                    # left image edge: y2[-1] := y2[0]
                    nc.gpsimd.tensor_tensor(up_qq[:, :, 0:1], sy3[:, :, 0:1],
                                            sy3[:, :, 0:1], Alu.add)
                else:
                    pwe = PWE[p - 1]
                    psy3 = state[("sy", p - 1)][:, 0:RPU * pwe].rearrange(
                        "q (r c) -> q r c", r=RPU)
                    nc.gpsimd.tensor_tensor(up_qq[:, :, 0:1],
                                            psy3[:, :, pwe - 1:pwe],
                                            sy3[:, :, 0:1], Alu.add)
            # O_er[17,pw] = y2[k] + y2[k+1]
            eng = nc.gpsimd if UPS_ENG[1] == "p" and not lastp else nc.vector
            if ov:
                eng.tensor_tensor(
                    upt[:, oer:oee].rearrange("q (r c) -> q r c", r=RPB),
                    sy3[:, 0:RPB, 1:we], sy3[:, 1:RPU, 1:we], Alu.add)
            else:
                eng.tensor_tensor(upt[:, oer:oee], syt[:, 0:RPB * pw],
                                  syt[:, pw:uc], Alu.add)
            # O_rc[17,pw] = 2 * y2[1:18]
            eng = nc.gpsimd if UPS_ENG[3] == "p" else nc.vector
            if ov:
                eng.tensor_scalar(
                    upt[:, orc:oqq].rearrange("q (r c) -> q r c", r=RPB),
                    sy3[:, 1:RPU, 1:we], 2.0, None, Alu.mult)
            else:
                eng.tensor_scalar(upt[:, orc:oqq], syt[:, pw:uc], 2.0, None,
                                  Alu.mult)

        def s_ups2(p):
            pw = PPW[p]
            oqq, oer, oee, oend = (RPB * pw, (RPB + RPU) * pw,
                                   (2 * RPB + RPU) * pw, (3 * RPB + RPU) * pw)
            upt = state[("up", p)]
            # O_ee[17,pw] = QQ[k] + QQ[k+1] (4x the logit)
            eng = (nc.gpsimd if UPS_ENG[2] == "p" and p != NPH - 1
                   else nc.vector)
            eng.tensor_tensor(upt[:, oee:oend], upt[:, oqq:oqq + RPB * pw],
                              upt[:, oqq + pw:oer], Alu.add)
            # sigmoids over [O_rc] and [QQ|O_er] (all hold 2x the logit);
            # split so Act's in-order queue never blocks evictions for long
            fot = fop.tile([128, max(PSTRIP)], bf16, tag="fo", name="fot")
            # split point: halve the [O_rc|QQ|O_er] range so neither piece
            # head-of-line-blocks Act's queue for long (sigmoid is
            # elementwise, any column split is valid)
            osp = (oee // (2 * pw)) * pw if SIG_HALF else oqq
            if SIG_MERGE:
                nc.scalar.activation(fot[:, 0:oee], upt[:, 0:oee],
                                     Act.Sigmoid, bias=b2ap, scale=0.5)
            else:
                nc.scalar.activation(fot[:, 0:osp], upt[:, 0:osp],
                                     Act.Sigmoid, bias=b2ap, scale=0.5)
            state[("fo", p)] = fot

        def s_ups2b(p):
            pw = PPW[p]
            oqq, oee = RPB * pw, (2 * RPB + RPU) * pw
            osp = (oee // (2 * pw)) * pw if SIG_HALF else oqq
            upt = state[("up", p)]
            fot = state[("fo", p)]
            if not SIG_MERGE:
                nc.scalar.activation(fot[:, osp:oee], upt[:, osp:oee],
                                     Act.Sigmoid, bias=b2ap, scale=0.5)
            dst = outd[:, :, OOFF[p]:OOFF[p] + oee]
            # last phase: use the scalar queue so this dispatch overlaps the
            # final O_ee DMA's on the sync queue (nothing left on Act after)
            q = nc.scalar if (p == NPH - 1 and LASTQ) else nc.sync
            q.dma_start(dst.rearrange("g i v -> (g i) v"), fot[:, 0:oee])

        def s_ups3(p):
            pw = PPW[p]
            oee, oend = (2 * RPB + RPU) * pw, (3 * RPB + RPU) * pw
            upt = state.pop(("up", p))
            fot = state.pop(("fo", p))
            # O_ee holds 4x the logit
            nc.scalar.activation(fot[:, oee:oend], upt[:, oee:oend],
                                 Act.Sigmoid, bias=b2ap, scale=0.25)
            dst = outd[:, :, OOFF[p] + oee:OOFF[p + 1]]
            nc.sync.dma_start(dst.rearrange("g i v -> (g i) v"),
                              fot[:, oee:oend])

        ups_q = []  # [phase, mm2-done tick, next stage]
        TOTAL = N_UNITS + D_MM2 + U3 + 2
        OLDF = os.environ.get("K_OLDF", "0") == "1"
        for i in range(TOTAL):
            if OLDF:
                j = i - D_MM2
                if 0 <= j < N_UNITS:
                    s_mm2(j)
                    if j % BLK == BLK - 1:
                        ups_q.append([j // BLK, i, 1])
                j = i - D_EV1
                if 0 <= j < N_UNITS:
                    s_ev1(j)
                j = i - D_MM1
                if 0 <= j < N_UNITS:
                    s_mm1(j)
                j = i - D_EV0
                if 0 <= j < N_UNITS and j % 2 == 0:
                    s_ev0(j)
                if i < N_UNITS and i % 2 == 0:
                    s_mm0(i)
            else:
                if i < N_UNITS and i % 2 == 0:
                    s_mm0(i)
                j = i - D_EV0
                if 0 <= j < N_UNITS and j % 2 == 0:
                    s_ev0(j)
                j = i - D_MM1
                if 0 <= j < N_UNITS:
                    s_mm1(j)
                j = i - D_EV1
                if 0 <= j < N_UNITS:
                    s_ev1(j)
                j = i - D_MM2
                if 0 <= j < N_UNITS:
                    s_mm2(j)
                    if j % BLK == BLK - 1:
                        ups_q.append([j // BLK, i, 1])
            last = i >= TOTAL - 1
            # once every matmul is emitted there is no PE pipeline left to
            # protect from Act head-of-line blocking: flush the remaining
            # upsample stages with tight spacing to shorten the drain tail
            drain = i >= N_UNITS + D_MM2 + int(os.environ.get("K_DREL", "99"))
            UTAIL = int(os.environ.get("K_UTAIL", "99"))
            for item in list(ups_q):
                p, t, st = item
                done = i - t
                if drain or p >= NPH - 2:
                    u2, u2b, u3 = ((2, 3, 4) if drain
                                   else (min(U2, UTAIL), min(U2 + 2, UTAIL + 2),
                                         min(U3, UTAIL + 4)))
                else:
                    u2, u2b, u3 = U2, U2 + 2, U3
                if st == 1 and (done >= U1 or last):
                    s_ups1(p)
                    item[2] = st = 2
                if st == 2 and (done >= u2 or last):
                    s_ups2(p)
                    item[2] = st = 3
                if st == 3 and (done >= u2b or last):
                    s_ups2b(p)
                    item[2] = st = 4
                if st == 4 and (done >= u3 or last):
                    s_ups3(p)
                    ups_q.remove(item)

    nc.compile()
    return nc


def _host_prep(mask_feats, mask_head_params, locations, im_inds, fpn_levels,
               sizes_of_interest):
    import ml_dtypes
    bf16 = ml_dtypes.bfloat16

    mask_feats = np.asarray(mask_feats, dtype=np.float32)
    params = np.asarray(mask_head_params, dtype=np.float32)
    locations = np.asarray(locations, dtype=np.float32)
    im_inds = np.asarray(im_inds).astype(np.int64)
    soi_tab = np.asarray(sizes_of_interest, dtype=np.float32)
    fpn_levels = np.asarray(fpn_levels).astype(np.int64)

    w0 = params[:, 0:80].reshape(N_INST, CH, CIN + 2)
    w1 = params[:, 80:144].reshape(N_INST, CH, CH)
    w2 = params[:, 144:152].reshape(N_INST, 1, CH)
    b0 = params[:, 152:160]
    b1 = params[:, 160:168]
    b2 = params[:, 168:169]

    soi = soi_tab[fpn_levels]
    alpha = -w0[:, :, 0] / soi[:, None]
    beta = -w0[:, :, 1] / soi[:, None]
    c0 = b0 + (w0[:, :, 0] * locations[:, 0:1]
               + w0[:, :, 1] * locations[:, 1:2]) / soi[:, None]
    wfeat = w0[:, :, 2:]

    stride = 8
    xs = np.arange(W, dtype=np.float32) * stride + stride // 2
    ys = np.arange(H, dtype=np.float32) * stride + stride // 2
    z3 = np.empty((K0, H, W), np.float32)
    z3[0] = xs[None, :]
    z3[1] = ys[:, None]
    z3[2] = 1.0
    z3[3:] = mask_feats.reshape(N_IMG * CIN, H, W)
    # strips in PROCESSING order; each strip leads with a duplicate of
    # image row 0 (block 0's edge-pad halo row); the first-processed strip
    # also carries its left-overlap column
    zb = z3[:, np.concatenate([[0], np.arange(H)]), :]        # (K0, 137, W)
    coff = np.cumsum([0] + PHW).tolist()
    strips = []
    for k in range(NPH):
        p = PORD[k]
        s = zb[:, :, coff[p] - POV[k]:coff[p + 1]]            # (K0, 137, we)
        strips.append(s.reshape(K0, (H + 1) * PWE[k]))
    z = np.ascontiguousarray(np.concatenate(strips, axis=1)).astype(bf16)

    in_maps = []
    for c in range(N_CORES):
        a0 = np.zeros((K0, 128), np.float32)
        w1p = np.zeros((128, 128), np.float32)
        w2p = np.zeros((128, 368), np.float32)
        bbv = np.zeros((128, 2), np.float32)
        for i in range(IPC):
            gi = IPC * c + i
            for o in range(CH):
                m = CH * i + o
                a0[0, m] = alpha[gi, o]
                a0[1, m] = beta[gi, o]
                a0[2, m] = c0[gi, o]
                base = 3 + CIN * int(im_inds[gi])
                a0[base:base + CIN, m] = wfeat[gi, o, :]
                w1p[CH * i:CH * i + CH, m] = w1[gi, o, :]
                bbv[m, 0] = b1[gi, o]
            w2p[CH * i:CH * i + CH, 240 + i] = w2[gi, 0, :]
        for q in range(128):
            bbv[q, 1] = b2[IPC * c + (q % IPC), 0]
        in_maps.append({
            "z_in": z,
            "a0t_in": a0.astype(bf16),
            "w1_in": w1p.astype(bf16),
            "w2_in": w2p.astype(bf16),
            "b_in": bbv,
        })
    return in_maps


def kernel(mask_feats, mask_head_params, locations, im_inds, fpn_levels,
           sizes_of_interest, mask_feat_stride):
    global LAST_EXEC_TIME_NS
    assert int(mask_feat_stride) == 8, "kernel hardcodes mask_feat_stride=8"

    os.environ["BASS_NEVER_TRACE"] = "1"
    from concourse.bass_utils import run_bass_kernel_spmd

    in_maps = _host_prep(mask_feats, mask_head_params, locations, im_inds,
                         fpn_levels, sizes_of_interest)

    if "nc" not in _CACHE:
        _CACHE["nc"] = _build_program()
    nc = _CACHE["nc"]

    res = run_bass_kernel_spmd(nc, in_maps, list(range(N_CORES)), trace=False)
    LAST_EXEC_TIME_NS = res.exec_time_ns

    coff = np.cumsum([0] + PHW).tolist()
    out = np.empty((N_INST, 1, OH, OW), np.float32)
    for c in range(N_CORES):
        dev = np.asarray(res.results[c]["out"]).astype(np.float32)
        # dev: [blk g, inst i, strip cols]
        o6 = np.empty((IPC, BLK, RPB, 2, OW), np.float32)
        for k in range(NPH):
            p = PORD[k]
            pw = PHW[p]
            orc, oqq, oer, oee = (np.array([0, RPB, RPB + RPU, 2 * RPB + RPU])
                                  * pw + OOFF[k])
            def plane(off, r):
                pl = dev[:, :, off:off + r * pw].reshape(BLK, IPC, r, pw)
                return pl.transpose(1, 0, 2, 3)
            c0_, c1_ = 2 * coff[p], 2 * coff[p + 1]
            o6[:, :, :, 1, c0_ + 1:c1_:2] = plane(orc, RPB)
            o6[:, :, :, 1, c0_:c1_:2] = plane(oqq, RPU)[:, :, 1:, :]
            o6[:, :, :, 0, c0_ + 1:c1_:2] = plane(oer, RPB)
            o6[:, :, :, 0, c0_:c1_:2] = plane(oee, RPB)
        out[IPC * c:IPC * (c + 1), 0] = o6.transpose(0, 1, 2, 3, 4).reshape(
            IPC, OH, OW)
    return out


# revision 41
# speedup vs baseline: 1.1954x; 1.0018x over previous
"""
Trainium2 Bass kernel for CondConv mask head (CondInst-style dynamic mask head).

Computation (fixed problem size):
  mask_feats (2, 8, 136, 200), 128 instances with per-instance 169 params
  -> per-instance 3-layer 1x1 convs over [rel_coords(2); feats(8)] -> (128,1,136,200)
  -> aligned_bilinear x2 upsample -> sigmoid -> (128, 1, 272, 400)

Strategy (8 NeuronCores, 16 instances per core), v2:
  * All matmul operands are bf16 (1 PE cycle/col).  Host folds rel-coords
    into a shared 19-row spatial matrix Z = [x; y; 1; feats_im0; feats_im1]
    and per-core lhsTs (a0 with the c0 constant on the ones-row,
    block-diagonal w1, and a zero-padded w2 strip whose eight 128-wide
    windows place the w2 block at lhsT columns 16g).
  * The image is processed in 8 column-phases of tunable widths (default
    20,28x6,12 -- narrow last phase to shorten the drain tail); each phase
    is 8 row-block units.  Row blocks OVERLAP by one row (18 rows, width*18
    cols <= 504): block g covers image rows 17g-1 .. 17g+16 (block 0
    edge-pads by duplicating row 0 in Z), so the x2-upsample top halo row
    is computed locally and no cross-partition halo DMA is needed.
  * mm0 writes unit PAIRS into a 2-bank PSUM tile (outs at col 0 and 512 so
    each matmul stays within a bank); the pair is evicted with one relu
    instruction (2D access pattern).  mm1/evict are per-unit; mm2
    accumulates phase logits into py2[128 = 8 blk x 16 inst].  PSUM-reading
    evictions run on ScalarE/VectorE only (GPSIMD has no PSUM port); the
    Act/DVE split is schedule-tuned.  PSUM: p0 2x2 banks + p1 2 + py2 2 = 8.
  * Upsample = 4 polyphase planes, packed bf16 in SBUF (VectorE 2x/4x
    modes, GpSimd helps off the critical chain):
      sy   = py2 copy (packed [18 x W'])
      O_rc = 2*sy[1:18]                 (odd row, odd col)
      QQ   = sy[.,j-1]+sy[.,j] (18 rows; rows 1..17 are the O_re plane;
             col 0 reads the previous phase's last col, tiny separate op)
      O_er = sy[k]+sy[k+1]              (even row, odd col)
      O_ee = QQ[k]+QQ[k+1]              (even row, even col, 4x logit)
  * Sigmoids: one Act instruction over [O_rc|QQ|O_er] (all 2x the logit:
    scale 0.5, bias b2) and one over O_ee (scale 0.25), emitted several
    units later so Act's in-order queue never head-blocks on the planes ->
    contiguous bf16 fo tile -> one 128-descriptor DMA per phase into
    [blk, inst, phase-strip] DRAM.  Host interleaves the planes, f32-casts.
"""

import os
import numpy as np

CH = 8
CIN = 8
N_IMG, H, W = 2, 136, 200
HW = H * W
N_INST = 128
N_CORES = 8
IPC = 16                         # instances per core
FACTOR = 2
OH, OW = H * FACTOR, W * FACTOR  # 272, 400
BLK = 8                          # row-blocks (= units) per phase
RPB = H // BLK                   # 17 output rows per block
RPU = RPB + 1                    # 18 stored rows per unit (one overlap row)
K0 = 3 + N_IMG * CIN             # 19 contraction rows for layer 0

PHW = [int(x) for x in os.environ.get(
    "K_PHW", "6,26,28,28,28,28,28,28").split(",")]
NPH = len(PHW)
assert sum(PHW) == W and all(w <= 28 for w in PHW)
# processing order: rotate so a narrow phase is processed LAST (short drain
# tail).  The first-processed phase carries one extra leading z column (its
# left overlap); every other phase reads its left neighbour's sy, which the
# rotation guarantees was processed just before (spatial phase 0 edge-pads).
ROT = int(os.environ.get("K_ROT", "0")) % NPH
PORD = [(ROT + k) % NPH for k in range(NPH)]
PPW = [PHW[p] for p in PORD]              # width per processing position
POV = [1 if (k == 0 and PORD[0] != 0) else 0 for k in range(NPH)]
PWE = [w + o for w, o in zip(PPW, POV)]   # effective (stored) width
assert all(w <= 28 for w in PWE)
# unit 0 of each phase has 18 rows (leading duplicate of image row 0, the
# edge-pad halo for block 0's 18-row mm2 window)
ZOFF = np.cumsum([0] + [(H + 1) * w for w in PWE]).tolist()
ZHW = ZOFF[-1]
PSTRIP = [(3 * RPB + RPU) * w for w in PPW]   # 69*w out cols per position
OOFF = np.cumsum([0] + PSTRIP).tolist()
N_UNITS = NPH * BLK

N_WARM = int(os.environ.get("K_WARM", "3"))
EV0A = int(os.environ.get("K_EV0A", "29"))   # of 32 pair evicts (y0) on Act
EV1A = int(os.environ.get("K_EV1A", "0"))   # of 64 single evicts (y1) on Act

LAST_EXEC_TIME_NS = None
_CACHE = {}


def _spread(n_act, total, skip=int(os.environ.get("K_SKIP0", "1"))):
    # spread n_act picks over [skip, total) so the first evictions (pipeline
    # fill, before Act's activation tables are loaded) go to DVE
    if n_act <= 0:
        return set()
    n_act = min(n_act, total - skip)
    return set((skip + np.arange(n_act) * (total - skip) // n_act).tolist())


def _build_program():
    import concourse.bass as bass
    import concourse.bacc as bacc
    import concourse.tile as tile
    from concourse import mybir
    from contextlib import ExitStack

    f32 = mybir.dt.float32
    bf16 = mybir.dt.bfloat16
    Alu = mybir.AluOpType
    Act = mybir.ActivationFunctionType

    nc = bacc.Bacc("TRN2", target_bir_lowering=False, debug=False)

    zd = nc.dram_tensor("z_in", [K0, ZHW], bf16, kind="ExternalInput").ap()
    a0d = nc.dram_tensor("a0t_in", [K0, 128], bf16, kind="ExternalInput").ap()
    w1d = nc.dram_tensor("w1_in", [128, 128], bf16, kind="ExternalInput").ap()
    w2d = nc.dram_tensor("w2_in", [128, 368], bf16, kind="ExternalInput").ap()
    bd = nc.dram_tensor("b_in", [128, 2], f32, kind="ExternalInput").ap()
    outd = nc.dram_tensor("out", [BLK, IPC, OOFF[-1]], bf16,
                          kind="ExternalOutput").ap()

    EV1F = int(os.environ.get("K_EV1F", "0"))
    pat = os.environ.get("K_EV0PAT", "0,4,20,28")
    if pat:
        dve_pairs = set(int(x) for x in pat.split(",") if x != "")
        ACT0 = set(range(N_UNITS // 2)) - dve_pairs
    else:
        ACT0 = _spread(EV0A, N_UNITS // 2)
    ACT1 = _spread(EV1A, N_UNITS, skip=5) | set(range(2, 2 + EV1F))

    U1 = int(os.environ.get("K_U1", "1"))
    U2 = int(os.environ.get("K_U2", "11"))
    U3 = int(os.environ.get("K_U3", "14"))
    D_EV0 = int(os.environ.get("K_DEV0", "1"))
    D_MM1 = int(os.environ.get("K_DMM1", "3"))
    D_EV1 = int(os.environ.get("K_DEV1", "4"))
    D_MM2 = int(os.environ.get("K_DMM2", "8"))
    # engines for [QQ, O_er, O_ee, O_rc]: p=pool, d=dve
    UPS_ENG = os.environ.get("K_UPS", "pppd")
    SY_ACT = os.environ.get("K_SY", "act") == "act"
    SIG_MERGE = os.environ.get("K_SIGM", "0") == "1"
    SIG_HALF = os.environ.get("K_SIGH", "0") == "1"
    SY_DVE_P = set(int(x) for x in os.environ.get("K_SYP", "").split(",")
                   if x != "")
    LASTQ = os.environ.get("K_LASTQ", "0") == "1"

    with tile.TileContext(nc) as tc, ExitStack() as ctx:
        consts = ctx.enter_context(tc.tile_pool(name="consts", bufs=1))
        zt = consts.tile([K0, ZHW], bf16)
        a0 = consts.tile([K0, 128], bf16)
        w1 = consts.tile([128, 128], bf16)
        w2s = consts.tile([128, 368], bf16)
        bb = consts.tile([128, 2], f32)
        warm = consts.tile([K0, 512], bf16)
        scr = consts.tile([1, 8], f32)

        # activation-table preloads first: tiny memset, then one dummy
        # activation per function so the table loads absorb into the idle
        # start instead of blocking the first Act evictions
        nc.vector.memset(scr[:], 0.0)
        nc.scalar.activation(scr[:], scr[:], Act.Sigmoid)
        nc.scalar.activation(scr[:], scr[:], Act.Relu)
        nc.scalar.activation(scr[:], scr[:], Act.Identity)
        nc.vector.memset(warm[:], 0.0)

        # z for the first unit-pair first (mm0 starts ASAP), then consts,
        # then the rest of z
        DORD = os.environ.get("K_DORD", "z0,a0,bb,w1,w2,zm,zt")
        dmas = {
            "z0": lambda: nc.sync.dma_start(zt[:, 0:ZOFF[1]], zd[:, 0:ZOFF[1]]),
            "z1": lambda: nc.sync.dma_start(zt[:, ZOFF[1]:ZOFF[2]],
                                            zd[:, ZOFF[1]:ZOFF[2]]),
            "a0": lambda: nc.sync.dma_start(a0[:], a0d),
            "bb": lambda: nc.sync.dma_start(bb[:], bd),
            "w1": lambda: nc.sync.dma_start(w1[:], w1d),
            "w2": lambda: nc.sync.dma_start(w2s[:], w2d),
            "zm": lambda: nc.sync.dma_start(zt[:, ZOFF[1]:ZOFF[4]],
                                            zd[:, ZOFF[1]:ZOFF[4]]),
            "z2": lambda: nc.sync.dma_start(zt[:, ZOFF[2]:ZOFF[4]],
                                            zd[:, ZOFF[2]:ZOFF[4]]),
            "zt": lambda: nc.sync.dma_start(zt[:, ZOFF[4]:], zd[:, ZOFF[4]:]),
        }
        for kk in DORD.split(","):
            dmas[kk]()

        b1ap = bb[:, 0:1]
        b2ap = bb[:, 1:2]
        w2g = [w2s[:, 240 - 16 * g:368 - 16 * g] for g in range(BLK)]

        p0p = ctx.enter_context(tc.tile_pool(name="p0p", bufs=2, space="PSUM"))
        p1p = ctx.enter_context(tc.tile_pool(name="p1p", bufs=2, space="PSUM"))
        py2p = ctx.enter_context(tc.tile_pool(name="py2p", bufs=2, space="PSUM"))
        y0p = ctx.enter_context(tc.tile_pool(name="y0p", bufs=int(os.environ.get("K_Y0B", "3"))))
        y1p = ctx.enter_context(tc.tile_pool(name="y1p", bufs=2))
        syp = ctx.enter_context(tc.tile_pool(name="syp", bufs=3))
        upp = ctx.enter_context(tc.tile_pool(name="upp", bufs=3))
        fop = ctx.enter_context(tc.tile_pool(name="fop", bufs=3))

        # PE warm-up while the first z chunk's DMA is in flight
        for wi in range(N_WARM):
            pw = p0p.tile([128, 1024], f32, tag="p0", name="pw")
            nc.tensor.matmul(pw[:, 0:512], warm[:, 0:128], warm[:],
                             start=True, stop=True)

        state = {}

        def evict(on_act, dst, src, bias_ap):
            if on_act:
                if bias_ap is None:
                    nc.scalar.activation(dst, src, Act.Relu)
                else:
                    nc.scalar.activation(dst, src, Act.Relu, bias=bias_ap)
            else:
                if bias_ap is None:
                    nc.vector.tensor_scalar(dst, src, 0.0, None, Alu.max)
                else:
                    nc.vector.tensor_scalar(dst, src, bias_ap, 0.0,
                                            Alu.add, Alu.max)

        def ucols(p, g):
            # unit 0 of a phase carries the extra duplicate row (18 rows)
            return (RPB + (1 if g == 0 else 0)) * PWE[p]

        def uoff(p, g):
            return ZOFF[p] + (RPB * g + (1 if g else 0)) * PWE[p]

        def s_mm0(u):
            p, g = divmod(u, BLK)
            q = u // 2
            ua, ub = ucols(p, g), ucols(p, g + 1)
            p0t = p0p.tile([128, 1024], f32, tag="p0", name="p0t")
            nc.tensor.matmul(p0t[:, 0:ua], a0[:],
                             zt[:, uoff(p, g):uoff(p, g) + ua],
                             start=True, stop=True)
            nc.tensor.matmul(p0t[:, 512:512 + ub], a0[:],
                             zt[:, uoff(p, g + 1):uoff(p, g + 1) + ub],
                             start=True, stop=True)
            state[("p0", q)] = p0t

        def s_ev0(u):
            p, g = divmod(u, BLK)
            ua, ub = ucols(p, g), ucols(p, g + 1)
            q = u // 2
            p0t = state.pop(("p0", q))
            y0t = y0p.tile([128, 1024], bf16, tag="y0", name="y0t")
            if ua == ub:
                src2 = p0t[:].rearrange("p (b c) -> p b c", b=2)[:, :, 0:ua]
                dst2 = y0t[:].rearrange("p (b c) -> p b c", b=2)[:, :, 0:ua]
                evict(q in ACT0, dst2, src2, None)
            elif 512 - ub <= int(os.environ.get("K_J0", "150")):
                # unequal halves (18-row unit 0): one 1D evict across both
                # banks, the small inter-bank gap is junk
                evict(q in ACT0, y0t[:, 0:512 + ub], p0t[:, 0:512 + ub], None)
            else:
                # narrow phase: the gap would dwarf the data; evict each
                # bank separately
                evict(q in ACT0, y0t[:, 0:ua], p0t[:, 0:ua], None)
                evict(q in ACT0, y0t[:, 512:512 + ub], p0t[:, 512:512 + ub],
                      None)
            state[("y0", q)] = y0t

        def s_mm1(u):
            p, g = divmod(u, BLK)
            uc = ucols(p, g)
            q, half = divmod(u, 2)
            y0t = state[("y0", q)]
            if half == 1:
                del state[("y0", q)]
            p1t = p1p.tile([128, 512], f32, tag="p1", name="p1t")
            nc.tensor.matmul(p1t[:, 0:uc], w1[:],
                             y0t[:, half * 512:half * 512 + uc],
                             start=True, stop=True)
            state[("p1", u)] = p1t

        def s_ev1(u):
            p, g = divmod(u, BLK)
            pw = PWE[p]
            uc = ucols(p, g)
            p1t = state.pop(("p1", u))
            if g == 0:
                # per-phase y1 tile: slot 0 = the duplicated image row 0
                state[("y1ph", p)] = y1p.tile([128, (H + 1) * 28], bf16,
                                              tag="y1", name="y1t")
            y1t = state[("y1ph", p)]
            off = (RPB * g + (1 if g else 0)) * pw
            evict(u in ACT1, y1t[:, off:off + uc], p1t[:, 0:uc], b1ap)

        def s_mm2(u):
            p, g = divmod(u, BLK)
            pw = PWE[p]
            y1t = state[("y1ph", p)]
            if g == BLK - 1:
                del state[("y1ph", p)]
            if g == 0:
                state[("py2", p)] = py2p.tile([128, 512], f32, tag="py2",
                                              name="py2t")
            py2t = state[("py2", p)]
            # 18-row window: rows 17g-1 .. 17g+16 (slot 17g .. 17g+17)
            nc.tensor.matmul(py2t[:, 0:RPU * pw], w2g[g],
                             y1t[:, RPB * g * pw:(RPB * g + RPU) * pw],
                             start=(g == 0), stop=(g == BLK - 1),
                             skip_group_check=True)

        def s_ups1(p):
            pw = PPW[p]
            ov = POV[p]
            we = PWE[p]
            uc = RPU * we
            sp = PORD[p]                  # spatial phase index
            py2t = state.pop(("py2", p))
            # sy[18, we] = y2 logits (packed copy of py2)
            syt = syp.tile([128, 504], bf16, tag="sy", name="syt")
            sy3 = syt[:, 0:uc].rearrange("q (r c) -> q r c", r=RPU)
            if SY_ACT and p not in SY_DVE_P:
                nc.scalar.activation(syt[:, 0:uc], py2t[:, 0:uc], Act.Identity)
            else:
                nc.vector.tensor_copy(syt[:, 0:uc], py2t[:, 0:uc])
            state[("sy", p)] = syt
            if p >= 2:
                del state[("sy", p - 2)]
            # plane offsets in up/fo: [O_rc 17w][QQ 18w][O_er 17w][O_ee 17w]
            orc, oqq, oer, oee = (0, RPB * pw, (RPB + RPU) * pw,
                                  (2 * RPB + RPU) * pw)
            upt = upp.tile([128, max(PSTRIP)], bf16, tag="up", name="upt")
            state[("up", p)] = upt
            up_qq = upt[:, oqq:oer].rearrange("q (r c) -> q r c", r=RPU)
            lastp = p == NPH - 1
            # QQ[18,pw] = y2[., J-1] + y2[., J]; rows 1..17 are O_re.
            eng = nc.gpsimd if UPS_ENG[0] == "p" and not lastp else nc.vector
            if ov:
                # own leading overlap column: one full-width op
                eng.tensor_tensor(up_qq[:], sy3[:, :, 0:pw],
                                  sy3[:, :, 1:we], Alu.add)
            else:
                eng.tensor_tensor(up_qq[:, :, 1:pw], sy3[:, :, 0:pw - 1],
                                  sy3[:, :, 1:pw], Alu.add)
                if sp == 0:
                    # left image edge: y2[-1] := y2[0]
                    nc.gpsimd.tensor_tensor(up_qq[:, :, 0:1], sy3[:, :, 0:1],
                                            sy3[:, :, 0:1], Alu.add)
                else:
                    pwe = PWE[p - 1]
                    psy3 = state[("sy", p - 1)][:, 0:RPU * pwe].rearrange(
                        "q (r c) -> q r c", r=RPU)
                    nc.gpsimd.tensor_tensor(up_qq[:, :, 0:1],
                                            psy3[:, :, pwe - 1:pwe],
                                            sy3[:, :, 0:1], Alu.add)
            # O_er[17,pw] = y2[k] + y2[k+1]
            eng = nc.gpsimd if UPS_ENG[1] == "p" and not lastp else nc.vector
            if ov:
                eng.tensor_tensor(
                    upt[:, oer:oee].rearrange("q (r c) -> q r c", r=RPB),
                    sy3[:, 0:RPB, 1:we], sy3[:, 1:RPU, 1:we], Alu.add)
            else:
                eng.tensor_tensor(upt[:, oer:oee], syt[:, 0:RPB * pw],
                                  syt[:, pw:uc], Alu.add)
            # O_rc[17,pw] = 2 * y2[1:18]
            eng = nc.gpsimd if UPS_ENG[3] == "p" else nc.vector
            if ov:
                eng.tensor_scalar(
                    upt[:, orc:oqq].rearrange("q (r c) -> q r c", r=RPB),
                    sy3[:, 1:RPU, 1:we], 2.0, None, Alu.mult)
            else:
                eng.tensor_scalar(upt[:, orc:oqq], syt[:, pw:uc], 2.0, None,
                                  Alu.mult)

        def s_ups2(p):
            pw = PPW[p]
            oqq, oer, oee, oend = (RPB * pw, (RPB + RPU) * pw,
                                   (2 * RPB + RPU) * pw, (3 * RPB + RPU) * pw)
            upt = state[("up", p)]
            # O_ee[17,pw] = QQ[k] + QQ[k+1] (4x the logit)
            eng = (nc.gpsimd if UPS_ENG[2] == "p" and p != NPH - 1
                   else nc.vector)
            eng.tensor_tensor(upt[:, oee:oend], upt[:, oqq:oqq + RPB * pw],
                              upt[:, oqq + pw:oer], Alu.add)
            # sigmoids over [O_rc] and [QQ|O_er] (all hold 2x the logit);
            # split so Act's in-order queue never blocks evictions for long
            fot = fop.tile([128, max(PSTRIP)], bf16, tag="fo", name="fot")
            # split point: halve the [O_rc|QQ|O_er] range so neither piece
            # head-of-line-blocks Act's queue for long (sigmoid is
            # elementwise, any column split is valid)
            osp = (oee // (2 * pw)) * pw if SIG_HALF else oqq
            if SIG_MERGE:
                nc.scalar.activation(fot[:, 0:oee], upt[:, 0:oee],
                                     Act.Sigmoid, bias=b2ap, scale=0.5)
            else:
                nc.scalar.activation(fot[:, 0:osp], upt[:, 0:osp],
                                     Act.Sigmoid, bias=b2ap, scale=0.5)
            state[("fo", p)] = fot

        def s_ups2b(p):
            pw = PPW[p]
            oqq, oee = RPB * pw, (2 * RPB + RPU) * pw
            osp = (oee // (2 * pw)) * pw if SIG_HALF else oqq
            upt = state[("up", p)]
            fot = state[("fo", p)]
            if not SIG_MERGE:
                nc.scalar.activation(fot[:, osp:oee], upt[:, osp:oee],
                                     Act.Sigmoid, bias=b2ap, scale=0.5)
            dst = outd[:, :, OOFF[p]:OOFF[p] + oee]
            # last phase: use the scalar queue so this dispatch overlaps the
            # final O_ee DMA's on the sync queue (nothing left on Act after)
            q = nc.scalar if (p == NPH - 1 and LASTQ) else nc.sync
            q.dma_start(dst.rearrange("g i v -> (g i) v"), fot[:, 0:oee])

        def s_ups3(p):
            pw = PPW[p]
            oee, oend = (2 * RPB + RPU) * pw, (3 * RPB + RPU) * pw
            upt = state.pop(("up", p))
            fot = state.pop(("fo", p))
            # O_ee holds 4x the logit
            nc.scalar.activation(fot[:, oee:oend], upt[:, oee:oend],
                                 Act.Sigmoid, bias=b2ap, scale=0.25)
            dst = outd[:, :, OOFF[p] + oee:OOFF[p + 1]]
            nc.sync.dma_start(dst.rearrange("g i v -> (g i) v"),
                              fot[:, oee:oend])

        ups_q = []  # [phase, mm2-done tick, next stage]
        TOTAL = N_UNITS + D_MM2 + U3 + 2
        OLDF = os.environ.get("K_OLDF", "0") == "1"
        for i in range(TOTAL):
            if OLDF:
                j = i - D_MM2
                if 0 <= j < N_UNITS:
                    s_mm2(j)
                    if j % BLK == BLK - 1:
                        ups_q.append([j // BLK, i, 1])
                j = i - D_EV1
                if 0 <= j < N_UNITS:
                    s_ev1(j)
                j = i - D_MM1
                if 0 <= j < N_UNITS:
                    s_mm1(j)
                j = i - D_EV0
                if 0 <= j < N_UNITS and j % 2 == 0:
                    s_ev0(j)
                if i < N_UNITS and i % 2 == 0:
                    s_mm0(i)
            else:
                if i < N_UNITS and i % 2 == 0:
                    s_mm0(i)
                j = i - D_EV0
                if 0 <= j < N_UNITS and j % 2 == 0:
                    s_ev0(j)
                j = i - D_MM1
                if 0 <= j < N_UNITS:
                    s_mm1(j)
                j = i - D_EV1
                if 0 <= j < N_UNITS:
                    s_ev1(j)
                j = i - D_MM2
                if 0 <= j < N_UNITS:
                    s_mm2(j)
                    if j % BLK == BLK - 1:
                        ups_q.append([j // BLK, i, 1])
            last = i >= TOTAL - 1
            # once every matmul is emitted there is no PE pipeline left to
            # protect from Act head-of-line blocking: flush the remaining
            # upsample stages with tight spacing to shorten the drain tail
            drain = i >= N_UNITS + D_MM2 + int(os.environ.get("K_DREL", "99"))
            UTAIL = int(os.environ.get("K_UTAIL", "99"))
            for item in list(ups_q):
                p, t, st = item
                done = i - t
                if drain or p >= NPH - 2:
                    u2, u2b, u3 = ((2, 3, 4) if drain
                                   else (min(U2, UTAIL), min(U2 + 2, UTAIL + 2),
                                         min(U3, UTAIL + 4)))
                else:
                    u2, u2b, u3 = U2, U2 + 2, U3
                if st == 1 and (done >= U1 or last):
                    s_ups1(p)
                    item[2] = st = 2
                if st == 2 and (done >= u2 or last):
                    s_ups2(p)
                    item[2] = st = 3
                if st == 3 and (done >= u2b or last):
                    s_ups2b(p)
                    item[2] = st = 4
                if st == 4 and (done >= u3 or last):
                    s_ups3(p)
                    ups_q.remove(item)

    nc.compile()
    return nc


def _host_prep(mask_feats, mask_head_params, locations, im_inds, fpn_levels,
               sizes_of_interest):
    import ml_dtypes
    bf16 = ml_dtypes.bfloat16

    mask_feats = np.asarray(mask_feats, dtype=np.float32)
    params = np.asarray(mask_head_params, dtype=np.float32)
    locations = np.asarray(locations, dtype=np.float32)
    im_inds = np.asarray(im_inds).astype(np.int64)
    soi_tab = np.asarray(sizes_of_interest, dtype=np.float32)
    fpn_levels = np.asarray(fpn_levels).astype(np.int64)

    w0 = params[:, 0:80].reshape(N_INST, CH, CIN + 2)
    w1 = params[:, 80:144].reshape(N_INST, CH, CH)
    w2 = params[:, 144:152].reshape(N_INST, 1, CH)
    b0 = params[:, 152:160]
    b1 = params[:, 160:168]
    b2 = params[:, 168:169]

    soi = soi_tab[fpn_levels]
    alpha = -w0[:, :, 0] / soi[:, None]
    beta = -w0[:, :, 1] / soi[:, None]
    c0 = b0 + (w0[:, :, 0] * locations[:, 0:1]
               + w0[:, :, 1] * locations[:, 1:2]) / soi[:, None]
    wfeat = w0[:, :, 2:]

    stride = 8
    xs = np.arange(W, dtype=np.float32) * stride + stride // 2
    ys = np.arange(H, dtype=np.float32) * stride + stride // 2
    z3 = np.empty((K0, H, W), np.float32)
    z3[0] = xs[None, :]
    z3[1] = ys[:, None]
    z3[2] = 1.0
    z3[3:] = mask_feats.reshape(N_IMG * CIN, H, W)
    # strips in PROCESSING order; each strip leads with a duplicate of
    # image row 0 (block 0's edge-pad halo row); the first-processed strip
    # also carries its left-overlap column
    zb = z3[:, np.concatenate([[0], np.arange(H)]), :]        # (K0, 137, W)
    coff = np.cumsum([0] + PHW).tolist()
    strips = []
    for k in range(NPH):
        p = PORD[k]
        s = zb[:, :, coff[p] - POV[k]:coff[p + 1]]            # (K0, 137, we)
        strips.append(s.reshape(K0, (H + 1) * PWE[k]))
    z = np.ascontiguousarray(np.concatenate(strips, axis=1)).astype(bf16)

    in_maps = []
    for c in range(N_CORES):
        a0 = np.zeros((K0, 128), np.float32)
        w1p = np.zeros((128, 128), np.float32)
        w2p = np.zeros((128, 368), np.float32)
        bbv = np.zeros((128, 2), np.float32)
        for i in range(IPC):
            gi = IPC * c + i
            for o in range(CH):
                m = CH * i + o
                a0[0, m] = alpha[gi, o]
                a0[1, m] = beta[gi, o]
                a0[2, m] = c0[gi, o]
                base = 3 + CIN * int(im_inds[gi])
                a0[base:base + CIN, m] = wfeat[gi, o, :]
                w1p[CH * i:CH * i + CH, m] = w1[gi, o, :]
                bbv[m, 0] = b1[gi, o]
            w2p[CH * i:CH * i + CH, 240 + i] = w2[gi, 0, :]
        for q in range(128):
            bbv[q, 1] = b2[IPC * c + (q % IPC), 0]
        in_maps.append({
            "z_in": z,
            "a0t_in": a0.astype(bf16),
            "w1_in": w1p.astype(bf16),
            "w2_in": w2p.astype(bf16),
            "b_in": bbv,
        })
    return in_maps


def kernel(mask_feats, mask_head_params, locations, im_inds, fpn_levels,
           sizes_of_interest, mask_feat_stride):
    global LAST_EXEC_TIME_NS
    assert int(mask_feat_stride) == 8, "kernel hardcodes mask_feat_stride=8"

    os.environ["BASS_NEVER_TRACE"] = "1"
    from concourse.bass_utils import run_bass_kernel_spmd

    in_maps = _host_prep(mask_feats, mask_head_params, locations, im_inds,
                         fpn_levels, sizes_of_interest)

    if "nc" not in _CACHE:
        _CACHE["nc"] = _build_program()
    nc = _CACHE["nc"]

    res = run_bass_kernel_spmd(nc, in_maps, list(range(N_CORES)), trace=False)
    LAST_EXEC_TIME_NS = res.exec_time_ns

    coff = np.cumsum([0] + PHW).tolist()
    out = np.empty((N_INST, 1, OH, OW), np.float32)
    for c in range(N_CORES):
        dev = np.asarray(res.results[c]["out"]).astype(np.float32)
        # dev: [blk g, inst i, strip cols]
        o6 = np.empty((IPC, BLK, RPB, 2, OW), np.float32)
        for k in range(NPH):
            p = PORD[k]
            pw = PHW[p]
            orc, oqq, oer, oee = (np.array([0, RPB, RPB + RPU, 2 * RPB + RPU])
                                  * pw + OOFF[k])
            def plane(off, r):
                pl = dev[:, :, off:off + r * pw].reshape(BLK, IPC, r, pw)
                return pl.transpose(1, 0, 2, 3)
            c0_, c1_ = 2 * coff[p], 2 * coff[p + 1]
            o6[:, :, :, 1, c0_ + 1:c1_:2] = plane(orc, RPB)
            o6[:, :, :, 1, c0_:c1_:2] = plane(oqq, RPU)[:, :, 1:, :]
            o6[:, :, :, 0, c0_ + 1:c1_:2] = plane(oer, RPB)
            o6[:, :, :, 0, c0_:c1_:2] = plane(oee, RPB)
        out[IPC * c:IPC * (c + 1), 0] = o6.transpose(0, 1, 2, 3, 4).reshape(
            IPC, OH, OW)
    return out
